# revision 5
# baseline (speedup 1.0000x reference)
"""Trainium2 Bass kernel for the speech-enhancement loss function.

Math (matching the jax reference):
  loss_mag    = mean((clean_mag - enhan_mag)^2)
  d           = clean_pha - enhan_mag          (reference quirk: enhan_mag is phase_g)
  ip_loss     = mean(aw(d)),   aw(x) = |x - round(x/2pi)*2pi| = 2pi*|f|,
                f = q - round(q), q = d/2pi
  gd_loss     = mean(aw(gd)),  gd[:,0,:] = -d[:,0,:]; gd[:,j,:] = d[:,j-1,:]-d[:,j,:]
  iaf_loss    = mean(aw(iaf)), same shifted difference along the T axis
  cspc_loss   = mean(1 - cos(aw(d))) = mean(1 - cos(2pi f))
  loss_com    = mean((clean_com - enhan_com)^2) * 2
  loss_time   = mean(|clean_wav - enhan_wav|)
  loss_metric = mean((metric_g - 1)^2)            (tiny -> host)

Sharding: data-parallel over the batch dim, 2 batches per core on 8 cores.
Each core computes partial per-partition SUMS of each term into a [128, NCOLS]
accumulator, DMAed out whole; the host reduces partitions + cores.

Device pipeline (per phase tile, fp16 intermediates -- tolerance is 2e-2):
  d16 = cp - em (DVE), q16 = d*S, v16 = q + 1536, r16 = v - 1536 (exact fp16
  round-to-nearest of q), f16 = q - r in [-0.5, 0.5].
  ip:   af16 = |f| via tensor_scalar(abs_max 0) with fused accum (DVE 4x)
  cspc: sin(pi/2 - 2pi*af) via ACT Sin with accum
  gd:   y = W0 @ f16 on PE (fp16, 1 cyc/row); ACT Abs psum->g16; DVE
        ts2((g-0.5) abs_max 0) with accum = sum ||y|-0.5|
  iaf:  fd16 = shifted diff (DVE tt 2x); z16 = (|fd|-0.5) ts2 4x; DVE
        ts(abs_max 0) accum
  m2/c2: Pool subtract -> DVE stt self-mult with accum (1x, proven path)
  wav:  Pool subtract -> DVE ts(abs_max) accum
F-tiling uses OVERLAPPED tiles (rows 0:128 and 127:201): the second tile
recomputes row 127's f locally so its gd needs no cross-tile boundary patch;
its W0 has the j=0 column zeroed and all its accumulations skip row 0.
No final on-device reduction: the [128, NCOLS] acc is DMAed out directly.
Each accum_out instruction OVERWRITES its column, so every accumulation site
has a unique column (COLMAP).

Schedule: 4 phase passes first (their DVE/ACT work overlaps the later com/wav
DMA), com then wav trailing, with a small DVE-only final wav chunk so the
post-DMA tail is ~1us of compute + the fixed DMA-out epilogue. Predicted
~79us vs the 73.4us DMA-transfer floor (26.4 MB/core at 360 GB/s).
"""

import numpy as np

import concourse.bacc as bacc
import concourse.mybir as mybir
import concourse.tile as tile
from concourse.bass_utils import run_bass_kernel_spmd

F32 = mybir.dt.float32
F16 = mybir.dt.float16
OP = mybir.AluOpType
AF = mybir.ActivationFunctionType

B, F, T, L = 16, 201, 2048, 204800
NCORES = 8
BPC = B // NCORES  # batches per core

TWO_PI_64 = 2.0 * np.pi
S = float(np.float32(1.0) / np.float32(TWO_PI_64))  # 1/(2pi)
MAGIC = 1536.0  # 1.5 * 2^10: fp16 round-to-nearest-int trick
MAGIC32 = float(np.float32(1.5 * 2**23))  # fp32 magic (one-op ts2 variant)
HALF_PI = float(np.float32(np.pi / 2))
NEG_TWO_PI = float(np.float32(-TWO_PI_64))

# com per core: BPC*F*T*2 = 1646592 = 2 batches x (128 x 6432)
COM_ROWS, COM_COLS = 128, 6432
COM_CHUNK = 1608  # 4 chunks per batch
# wav per core: BPC*L = 409600 = 128 x 3200
WAV_ROWS, WAV_COLS = 128, 3200

NCOLS = 40  # accumulator columns (one per accumulation instruction)

# term -> list of acc columns, populated by build_nc (deterministic)
COLMAP = {}

# overlapped F tiles: (f0, P, lo) -- accumulate rows [lo:P] of the tile
FTILES = [(0, 128, 0), (127, 74, 1)]


def _w0_matrix(P, skip_first):
    # lhsT[k, j] = delta_{j,k+1} - delta_{j,k}  ->  (W0 @ f)[j] = f[j-1] - f[j]
    # skip_first: zero the j=0 column (row handled by the previous tile)
    w = np.zeros((P, P), dtype=np.float16)
    for k in range(P):
        w[k, k] = -1.0
        if k + 1 < P:
            w[k, k + 1] = 1.0
    if skip_first:
        w[0, 0] = 0.0
    return w


def build_nc(in_bufs=3, cp_bufs=3, wav_chunks=(1408, 1408, 384),
             m2_eng=("act", "act", "act", "act"),
             gd2_eng=("act", "act", "act", "act"),
             iaf2_eng=("dve", "dve", "dve", "dve"),
             com_eng=("act", "act", "act", "dve", "act", "act", "act", "dve"),
             wav_acc_eng=("dve", "dve", "dve"),
             magic1=False, last_wav_dve=True, qg_cols=1024,
             com_chunk=COM_CHUNK):
    CK = com_chunk
    nc = bacc.Bacc(None, target_bir_lowering=False)

    mag_c = nc.dram_tensor("mag_c", [BPC, F, T], F32, kind="ExternalInput")
    mag_e = nc.dram_tensor("mag_e", [BPC, F, T], F32, kind="ExternalInput")
    pha_c = nc.dram_tensor("pha_c", [BPC, F, T], F32, kind="ExternalInput")
    com_c = nc.dram_tensor("com_c", [BPC, COM_ROWS, COM_COLS], F32, kind="ExternalInput")
    com_e = nc.dram_tensor("com_e", [BPC, COM_ROWS, COM_COLS], F32, kind="ExternalInput")
    wav_c = nc.dram_tensor("wav_c", [WAV_ROWS, WAV_COLS], F32, kind="ExternalInput")
    wav_e = nc.dram_tensor("wav_e", [WAV_ROWS, WAV_COLS], F32, kind="ExternalInput")
    out_d = nc.dram_tensor("partials", [128, NCOLS], F32, kind="ExternalOutput")

    w0a_d = nc.inline_tensor(_w0_matrix(128, False), name="w0a")
    w0b_d = nc.inline_tensor(_w0_matrix(74, True), name="w0b")

    COLMAP.clear()
    _next_col = [0]

    def col(term, lo=0, hi=128):
        c = _next_col[0]
        _next_col[0] += 1
        assert c < NCOLS
        COLMAP.setdefault(term, []).append((c, lo, hi))
        return c

    with tile.TileContext(nc) as tc:
        with (
            tc.tile_pool(name="main", bufs=2) as pool,
            tc.tile_pool(name="psum", bufs=1, space="PSUM") as psum,
        ):
            acc = pool.tile([128, NCOLS], F32, tag="acc", bufs=1)
            nc.vector.memset(acc[:], 0.0)
            halfpi = pool.tile([128, 1], F32, tag="halfpi", bufs=1)
            nc.vector.memset(halfpi[:], HALF_PI)
            negh = pool.tile([128, 1], F32, tag="negh", bufs=1)
            nc.vector.memset(negh[:], -0.5)
            w0 = {}

            counters = {"pi": 0, "ci": 0, "wi": 0}

            def load_w0():
                w0a = pool.tile([128, 128], F16, tag="w0a", bufs=1)
                nc.sync.dma_start(w0a[:], w0a_d[:])
                w0b = pool.tile([74, 74], F16, tag="w0b", bufs=1)
                nc.sync.dma_start(w0b[:], w0b_d[:])
                w0[0] = w0a
                w0[127] = w0b

            def phase_pass(b, f0, P, lo):
                pi = counters["pi"]
                counters["pi"] += 1
                sl = slice(0, P)
                cm = pool.tile([P, T], F32, tag="in_a", bufs=in_bufs, name=f"cm{pi}")
                nc.sync.dma_start(cm[:], mag_c[b, f0 : f0 + P, :])
                em = pool.tile([P, T], F32, tag="in_b", bufs=in_bufs, name=f"em{pi}")
                nc.sync.dma_start(em[:], mag_e[b, f0 : f0 + P, :])
                if pi == 0:
                    load_w0()
                cp = pool.tile([P, T], F32, tag="in_c", bufs=cp_bufs, name=f"cp{pi}")
                nc.sync.dma_start(cp[:], pha_c[b, f0 : f0 + P, :])

                junk = pool.tile([P, T], F16, tag="junk", bufs=1, name=f"junk{pi}")

                # mag: m = cm - em (Pool), sum m^2 (ACT Square or DVE stt)
                m = pool.tile([P, T], F16, tag="m", name=f"m{pi}")
                nc.gpsimd.tensor_tensor(m[:], cm[:], em[:], OP.subtract)
                if m2_eng[pi] == "act":
                    nc.scalar.activation(
                        junk[sl, :], m[sl, :], AF.Square,
                        accum_out=acc[sl, (c := col("m2", lo, P)) : c + 1],
                    )
                else:
                    nc.vector.scalar_tensor_tensor(
                        junk[sl, :], m[sl, :], 0.0, m[sl, :], OP.bypass, OP.mult,
                        accum_out=acc[sl, (c := col("m2", lo, P)) : c + 1],
                    )

                # round chain: d -> q -> r = RNE(q) -> f = q - r (fp16)
                d = pool.tile([P, T], F16, tag="d", name=f"d{pi}")
                nc.vector.tensor_tensor(d[:], cp[:], em[:], OP.subtract)
                q = pool.tile([P, T], F16, tag="q", name=f"q{pi}")
                nc.vector.tensor_scalar(q[:], d[:], S, None, OP.mult)
                r = pool.tile([P, T], F16, tag="r", name=f"r{pi}")
                if magic1:
                    # one ts2: (q + 1.5*2^23) - 1.5*2^23 with fp32 internal RNE
                    nc.vector.tensor_scalar(r[:], q[:], MAGIC32, -MAGIC32, OP.add, OP.add)
                else:
                    v = pool.tile([P, T], F16, tag="v", name=f"v{pi}")
                    nc.vector.tensor_scalar(v[:], q[:], MAGIC, None, OP.add)
                    nc.vector.tensor_scalar(r[:], v[:], -MAGIC, None, OP.add)
                f = pool.tile([P, T], F16, tag="f", name=f"f{pi}")
                nc.vector.tensor_tensor(f[:], q[:], r[:], OP.subtract)

                # ip: af = |f| (ACT Abs, accum -> ip); cspc: ACT Sin(pi/2 - 2pi af)
                af = pool.tile([P, T], F16, tag="af", name=f"af{pi}")
                nc.scalar.activation(
                    af[sl, :], f[sl, :], AF.Abs,
                    accum_out=acc[sl, (c := col("ip", lo, P)) : c + 1],
                )
                nc.scalar.activation(
                    junk[sl, :], af[sl, :], AF.Sin, bias=halfpi[0:P, :],
                    scale=NEG_TWO_PI,
                    accum_out=acc[sl, (c := col("cos", lo, P)) : c + 1],
                )

                # gd: y = W0 @ f on PE; ACT Abs psum->g16; dist = ||y|-0.5| accum
                g = pool.tile([P, T], F16, tag="g", name=f"g{pi}")
                for h in range(0, T, qg_cols):
                    qg = psum.tile([P, qg_cols], F32, tag="qg", bufs=2, name=f"qg{pi}_{h}")
                    for n0 in range(0, qg_cols, 512):
                        nc.tensor.matmul(
                            qg[:, n0 : n0 + 512], w0[f0][0:P, 0:P],
                            f[:, h + n0 : h + n0 + 512],
                        )
                    nc.scalar.activation(g[sl, h : h + qg_cols], qg[sl, :], AF.Abs)
                if gd2_eng[pi] == "act":
                    nc.scalar.activation(
                        junk[sl, :], g[sl, :], AF.Abs, bias=negh[0:P, :],
                        accum_out=acc[sl, (c := col("gd", lo, P)) : c + 1],
                    )
                else:
                    zg = pool.tile([P, T], F16, tag="e", name=f"zg{pi}")
                    nc.vector.tensor_scalar(zg[sl, :], g[sl, :], -0.5, None, OP.add)
                    nc.vector.tensor_reduce(
                        acc[sl, (c := col("gd", lo, P)) : c + 1], zg[sl, :],
                        axis=mybir.AxisListType.X, op=OP.add,
                        apply_absolute_value=True,
                    )

                # iaf: fd = shifted diff of f; dist(fd) = |fd - RNE(fd)| accum
                fd = pool.tile([P, T], F16, tag="fd", name=f"fd{pi}")
                nc.vector.tensor_copy(fd[sl, 0:1], f[sl, 0:1])
                nc.vector.tensor_tensor(
                    fd[sl, 1:T], f[sl, 0 : T - 1], f[sl, 1:T], OP.subtract
                )
                u = pool.tile([P, T], F16, tag="r", name=f"u{pi}")
                if magic1:
                    nc.vector.tensor_scalar(u[sl, :], fd[sl, :], MAGIC32, -MAGIC32, OP.add, OP.add)
                else:
                    u1 = pool.tile([P, T], F16, tag="v", name=f"u1{pi}")
                    nc.vector.tensor_scalar(u1[sl, :], fd[sl, :], MAGIC, None, OP.add)
                    nc.vector.tensor_scalar(u[sl, :], u1[sl, :], -MAGIC, None, OP.add)
                e = pool.tile([P, T], F16, tag="e", name=f"e{pi}")
                nc.vector.tensor_tensor(e[sl, :], fd[sl, :], u[sl, :], OP.subtract)
                if iaf2_eng[pi] == "act":
                    nc.scalar.activation(
                        junk[sl, :], e[sl, :], AF.Abs,
                        accum_out=acc[sl, (c := col("iaf", lo, P)) : c + 1],
                    )
                else:
                    nc.vector.tensor_reduce(
                        acc[sl, (c := col("iaf", lo, P)) : c + 1], e[sl, :],
                        axis=mybir.AxisListType.X, op=OP.add,
                        apply_absolute_value=True,
                    )

            def com_pass(b, c0, cols):
                ci = counters["ci"]
                counters["ci"] += 1
                cc = pool.tile([COM_ROWS, cols], F32, tag="in_a", bufs=in_bufs, name=f"cc{ci}")
                nc.sync.dma_start(cc[:], com_c[b, :, c0 : c0 + cols])
                ec = pool.tile([COM_ROWS, cols], F32, tag="in_b", bufs=in_bufs, name=f"ec{ci}")
                nc.sync.dma_start(ec[:], com_e[b, :, c0 : c0 + cols])
                cd = pool.tile([COM_ROWS, cols], F16, tag="d", name=f"cd{ci}")
                nc.gpsimd.tensor_tensor(cd[:], cc[:], ec[:], OP.subtract)
                djunk = pool.tile([COM_ROWS, cols], F16, tag="junk", bufs=1, name=f"cj{ci}")
                if com_eng[ci] == "act":
                    nc.scalar.activation(
                        djunk[:], cd[:], AF.Square,
                        accum_out=acc[:, (c := col("c2")) : c + 1],
                    )
                else:
                    nc.vector.scalar_tensor_tensor(
                        djunk[:], cd[:], 0.0, cd[:], OP.bypass, OP.mult,
                        accum_out=acc[:, (c := col("c2")) : c + 1],
                    )

            def wav_pass(c0, cols, use_dve):
                wi = counters["wi"]
                counters["wi"] += 1
                cw = pool.tile([WAV_ROWS, cols], F32, tag="in_a", bufs=in_bufs, name=f"cw{wi}")
                nc.sync.dma_start(cw[:], wav_c[:, c0 : c0 + cols])
                ew = pool.tile([WAV_ROWS, cols], F32, tag="in_b", bufs=in_bufs, name=f"ew{wi}")
                nc.sync.dma_start(ew[:], wav_e[:, c0 : c0 + cols])
                wd = pool.tile([WAV_ROWS, cols], F16, tag="d", name=f"wd{wi}")
                if use_dve:
                    nc.vector.tensor_tensor(wd[:], cw[:], ew[:], OP.subtract)
                else:
                    nc.gpsimd.tensor_tensor(wd[:], cw[:], ew[:], OP.subtract)
                if wav_acc_eng[wi] == "act":
                    wjunk = pool.tile([WAV_ROWS, cols], F16, tag="junk", bufs=1, name=f"wj{wi}")
                    nc.scalar.activation(
                        wjunk[:], wd[:], AF.Abs,
                        accum_out=acc[:, (c := col("w")) : c + 1],
                    )
                else:
                    nc.vector.tensor_reduce(
                        acc[:, (c := col("w")) : c + 1], wd[:],
                        axis=mybir.AxisListType.X, op=OP.add,
                        apply_absolute_value=True,
                    )

            for b in range(BPC):
                for f0, P, lo in FTILES:
                    phase_pass(b, f0, P, lo)
            for b in range(BPC):
                for c0 in range(0, COM_COLS, CK):
                    com_pass(b, c0, CK)
            nchunks = len(wav_chunks)
            c0 = 0
            for i, cols in enumerate(wav_chunks):
                wav_pass(c0, cols, last_wav_dve and i == nchunks - 1)
                c0 += cols
            assert c0 == WAV_COLS

            nc.sync.dma_start(out_d[:], acc[:])

    nc.compile()
    return nc


_CACHE = {}


def _get_nc():
    if "nc" not in _CACHE:
        _CACHE["nc"] = build_nc()
    return _CACHE["nc"]


def make_in_maps(inputs):
    """Slice the full inputs into per-core input maps."""
    clean_mag = np.asarray(inputs["clean_mag"], dtype=np.float32)
    enhan_mag = np.asarray(inputs["enhan_mag"], dtype=np.float32)
    clean_pha = np.asarray(inputs["clean_pha"], dtype=np.float32)
    clean_com = np.asarray(inputs["clean_com"], dtype=np.float32)
    enhan_com = np.asarray(inputs["enhan_com"], dtype=np.float32)
    clean_wav = np.asarray(inputs["clean_wav"], dtype=np.float32)
    enhan_wav = np.asarray(inputs["enhan_wav"], dtype=np.float32)

    in_maps = []
    for i in range(NCORES):
        sl = slice(BPC * i, BPC * (i + 1))
        in_maps.append(
            {
                "mag_c": np.ascontiguousarray(clean_mag[sl]),
                "mag_e": np.ascontiguousarray(enhan_mag[sl]),
                "pha_c": np.ascontiguousarray(clean_pha[sl]),
                "com_c": np.ascontiguousarray(clean_com[sl]).reshape(
                    BPC, COM_ROWS, COM_COLS
                ),
                "com_e": np.ascontiguousarray(enhan_com[sl]).reshape(
                    BPC, COM_ROWS, COM_COLS
                ),
                "wav_c": np.ascontiguousarray(clean_wav[sl]).reshape(
                    WAV_ROWS, WAV_COLS
                ),
                "wav_e": np.ascontiguousarray(enhan_wav[sl]).reshape(
                    WAV_ROWS, WAV_COLS
                ),
            }
        )
    return in_maps


def combine(partials, inputs):
    """Combine per-core [128, NCOLS] partial sums into the 6 losses."""
    p = np.asarray(partials, dtype=np.float64)
    p = p.reshape(-1, 128, NCOLS)  # [cores, partitions, cols]

    def tsum(term):
        return sum(p[:, lo:hi, c].sum() for (c, lo, hi) in COLMAP[term])

    s_ip = tsum("ip")
    s_gd = tsum("gd")
    s_iaf = tsum("iaf")
    s_cos = tsum("cos")
    s_m2 = tsum("m2")
    s_c2 = tsum("c2")
    s_w = tsum("w")

    n = float(B * F * T)
    ip = TWO_PI_64 * s_ip / n
    # gd col holds sum(||y|-0.5|); dist(y) = 0.5 - ||y|-0.5|
    gd = TWO_PI_64 * (0.5 * n - s_gd) / n
    # iaf col holds sum(|fd - RNE(fd)|) = sum dist(fd) directly
    iaf = TWO_PI_64 * s_iaf / n
    cspc = 1.0 - s_cos / n
    loss_mag = s_m2 / n
    loss_pha = ip + gd + iaf + cspc
    loss_com = 2.0 * s_c2 / (n * 2.0)
    loss_time = s_w / float(B * L)

    metric_g = np.asarray(inputs["metric_g"], dtype=np.float64).reshape(-1)
    one_labels = np.asarray(inputs["one_labels"], dtype=np.float64).reshape(-1)
    loss_metric = float(np.mean((metric_g - one_labels) ** 2))

    nloss = (
        loss_mag * 0.9
        + loss_pha * 0.3
        + loss_com * 0.1
        + loss_metric * 0.05
        + loss_time * 0.2
    )
    return tuple(
        np.float32(x)
        for x in (nloss, loss_mag, loss_pha, loss_com, loss_metric, loss_time)
    )


def _get_runner():
    """Build (once) a persistently-compiled 8-core sharded executor.

    Mirrors bass2jax.run_bass_via_pjrt but caches the jitted function so
    repeat calls skip retracing/recompiling.
    """
    if "runner" in _CACHE:
        return _CACHE["runner"]
    import jax
    from concourse import bass2jax

    nc = _get_nc()
    bass2jax.install_neuronx_cc_hook()

    partition_name = nc.partition_id_tensor.name if nc.partition_id_tensor else None
    in_names, out_names, out_avals, zero_shapes = [], [], [], []
    for alloc in nc.m.functions[0].allocations:
        if not isinstance(alloc, mybir.MemoryLocationSet):
            continue
        name = alloc.memorylocations[0].name
        if alloc.kind == "ExternalInput":
            if name != partition_name:
                in_names.append(name)
        elif alloc.kind == "ExternalOutput":
            out_names.append(name)
            shape = tuple(alloc.tensor_shape)
            dtype = mybir.dt.np(alloc.dtype)
            out_avals.append(jax.core.ShapedArray(shape, dtype))
            zero_shapes.append((shape, dtype))
    n_params = len(in_names)
    all_in = list(in_names) + list(out_names)
    if partition_name is not None:
        all_in.append(partition_name)
    donate = tuple(range(n_params, n_params + len(out_names)))

    def _body(*args):
        operands = list(args)
        if partition_name is not None:
            operands.append(bass2jax.partition_id_tensor())
        outs = bass2jax._bass_exec_p.bind(
            *operands,
            out_avals=tuple(out_avals),
            in_names=tuple(all_in),
            out_names=tuple(out_names),
            lowering_input_output_aliases=(),
            sim_require_finite=True,
            sim_require_nnan=True,
            nc=nc,
        )
        return tuple(outs)

    devices = jax.devices()[:NCORES]
    mesh = bass2jax.Mesh(np.asarray(devices), ("core",))
    pspec = bass2jax.PartitionSpec("core")
    in_specs = (pspec,) * (n_params + len(out_names))
    out_specs = (pspec,) * len(out_names)
    sharded = jax.jit(
        bass2jax.shard_map(
            _body, mesh=mesh, in_specs=in_specs, out_specs=out_specs, check_rep=False
        ),
        donate_argnums=donate,
        keep_unused=True,
    )

    def make_zeros():
        return [
            np.zeros((NCORES * s[0], *s[1:]), d) for (s, d) in zero_shapes
        ]

    def call(concat_in):
        outs = sharded(*concat_in, *make_zeros())
        return np.asarray(outs[0]).reshape(NCORES, 128, NCOLS)

    def device_put(concat_in):
        sh = jax.sharding.NamedSharding(mesh, pspec)
        return [jax.device_put(a, sh) for a in concat_in]

    runner = (call, in_names, device_put, sharded, make_zeros)
    _CACHE["runner"] = runner
    return runner


def concat_inputs(in_maps, in_names):
    return [
        np.concatenate([m[name] for m in in_maps], axis=0) for name in in_names
    ]


def run(inputs):
    in_maps = make_in_maps(inputs)
    try:
        call, in_names, _, _, _ = _get_runner()
        partials = call(concat_inputs(in_maps, in_names))
    except Exception:
        nc = _get_nc()
        res = run_bass_kernel_spmd(nc, in_maps, core_ids=list(range(NCORES)))
        partials = np.asarray([r["partials"] for r in res.results])
    return combine(partials, inputs)


def kernel(**inputs):
    return run(inputs)


# revision 6
# speedup vs baseline: 1.1032x; 1.1032x over previous
"""Trainium2 Bass kernel for the speech-enhancement loss function.

Math (matching the jax reference):
  loss_mag    = mean((clean_mag - enhan_mag)^2)
  d           = clean_pha - enhan_mag          (reference quirk: enhan_mag is phase_g)
  ip_loss     = mean(aw(d)),   aw(x) = |x - round(x/2pi)*2pi| = 2pi*|f|,
                f = q - round(q), q = d/2pi
  gd_loss     = mean(aw(gd)),  gd[:,0,:] = -d[:,0,:]; gd[:,j,:] = d[:,j-1,:]-d[:,j,:]
  iaf_loss    = mean(aw(iaf)), same shifted difference along the T axis
  cspc_loss   = mean(1 - cos(aw(d))) = mean(1 - cos(2pi f))
  loss_com    = mean((clean_com - enhan_com)^2) * 2
  loss_time   = mean(|clean_wav - enhan_wav|)
  loss_metric = mean((metric_g - 1)^2)            (tiny -> host)

Sharding: data-parallel over the batch dim, 2 batches per core on 8 cores.
Each core computes partial per-partition SUMS of each term into a [128, NCOLS]
accumulator, DMAed out whole; the host reduces partitions + cores.

Device pipeline (per phase tile, fp16 intermediates -- tolerance is 2e-2):
  d16 = cp - em (DVE), q16 = d*S, v16 = q + 1536, r16 = v - 1536 (exact fp16
  round-to-nearest of q), f16 = q - r in [-0.5, 0.5].
  ip:   af16 = |f| via tensor_scalar(abs_max 0) with fused accum (DVE 4x)
  cspc: sin(pi/2 - 2pi*af) via ACT Sin with accum
  gd:   y = W0 @ f16 on PE (fp16, 1 cyc/row); ACT Abs psum->g16; DVE
        ts2((g-0.5) abs_max 0) with accum = sum ||y|-0.5|
  iaf:  fd16 = shifted diff (DVE tt 2x); z16 = (|fd|-0.5) ts2 4x; DVE
        ts(abs_max 0) accum
  m2/c2: Pool subtract -> DVE stt self-mult with accum (1x, proven path)
  wav:  Pool subtract -> DVE ts(abs_max) accum
F-tiling uses OVERLAPPED tiles (rows 0:128 and 127:201): the second tile
recomputes row 127's f locally so its gd needs no cross-tile boundary patch;
its W0 has the j=0 column zeroed and all its accumulations skip row 0.
No final on-device reduction: the [128, NCOLS] acc is DMAed out directly.
Each accum_out instruction OVERWRITES its column, so every accumulation site
has a unique column (COLMAP).

Schedule: 4 phase passes first (their DVE/ACT work overlaps the later com/wav
DMA), com then wav trailing, with a small DVE-only final wav chunk so the
post-DMA tail is ~1us of compute + the fixed DMA-out epilogue. Predicted
~79us vs the 73.4us DMA-transfer floor (26.4 MB/core at 360 GB/s).
"""

import numpy as np

import concourse.bacc as bacc
import concourse.mybir as mybir
import concourse.tile as tile
from concourse.bass_utils import run_bass_kernel_spmd

F32 = mybir.dt.float32
F16 = mybir.dt.float16
OP = mybir.AluOpType
AF = mybir.ActivationFunctionType

B, F, T, L = 16, 201, 2048, 204800
NCORES = 8
BPC = B // NCORES  # batches per core

TWO_PI_64 = 2.0 * np.pi
S = float(np.float32(1.0) / np.float32(TWO_PI_64))  # 1/(2pi)
MAGIC = 1536.0  # 1.5 * 2^10: fp16 round-to-nearest-int trick
MAGIC32 = float(np.float32(1.5 * 2**23))  # fp32 magic (one-op ts2 variant)
HALF_PI = float(np.float32(np.pi / 2))
NEG_TWO_PI = float(np.float32(-TWO_PI_64))

# com per core: BPC*F*T*2 = 1646592 = 2 batches x (128 x 6432)
COM_ROWS, COM_COLS = 128, 6432
COM_CHUNK = 1608  # 4 chunks per batch
# wav per core: BPC*L = 409600 = 128 x 3200
WAV_ROWS, WAV_COLS = 128, 3200

NCOLS = 40  # accumulator columns (one per accumulation instruction)

# term -> list of acc columns, populated by build_nc (deterministic)
COLMAP = {}

# overlapped F tiles: (f0, P, lo) -- accumulate rows [lo:P] of the tile
FTILES = [(0, 128, 0), (127, 74, 1)]


def _w0_matrix(P, skip_first):
    # lhsT[k, j] = delta_{j,k+1} - delta_{j,k}  ->  (W0 @ f)[j] = f[j-1] - f[j]
    # skip_first: zero the j=0 column (row handled by the previous tile)
    w = np.zeros((P, P), dtype=np.float16)
    for k in range(P):
        w[k, k] = -1.0
        if k + 1 < P:
            w[k, k + 1] = 1.0
    if skip_first:
        w[0, 0] = 0.0
    return w


def build_nc(in_bufs=3, cp_bufs=3, wav_chunks=(1408, 1408, 384),
             m2_eng=("act", "act", "act", "act"),
             gd2_eng=("act", "act", "act", "act"),
             iaf2_eng=("dve", "dve", "dve", "dve"),
             com_eng=("dve", "act", "dve", "act", "dve", "act", "dve", "act"),
             wav_acc_eng=("dve", "act", "dve"),
             magic1=True, last_wav_dve=True, qg_cols=1024,
             interleave=True, com_chunk=COM_CHUNK):
    CK = com_chunk
    nc = bacc.Bacc(None, target_bir_lowering=False)

    mag_c = nc.dram_tensor("mag_c", [BPC, F, T], F32, kind="ExternalInput")
    mag_e = nc.dram_tensor("mag_e", [BPC, F, T], F32, kind="ExternalInput")
    pha_c = nc.dram_tensor("pha_c", [BPC, F, T], F32, kind="ExternalInput")
    com_c = nc.dram_tensor("com_c", [BPC, COM_ROWS, COM_COLS], F32, kind="ExternalInput")
    com_e = nc.dram_tensor("com_e", [BPC, COM_ROWS, COM_COLS], F32, kind="ExternalInput")
    wav_c = nc.dram_tensor("wav_c", [WAV_ROWS, WAV_COLS], F32, kind="ExternalInput")
    wav_e = nc.dram_tensor("wav_e", [WAV_ROWS, WAV_COLS], F32, kind="ExternalInput")
    out_d = nc.dram_tensor("partials", [128, NCOLS], F32, kind="ExternalOutput")

    w0a_d = nc.inline_tensor(_w0_matrix(128, False), name="w0a")
    w0b_d = nc.inline_tensor(_w0_matrix(74, True), name="w0b")

    COLMAP.clear()
    _next_col = [0]

    def col(term, lo=0, hi=128):
        c = _next_col[0]
        _next_col[0] += 1
        assert c < NCOLS
        COLMAP.setdefault(term, []).append((c, lo, hi))
        return c

    with tile.TileContext(nc) as tc:
        with (
            tc.tile_pool(name="main", bufs=2) as pool,
            tc.tile_pool(name="psum", bufs=1, space="PSUM") as psum,
        ):
            acc = pool.tile([128, NCOLS], F32, tag="acc", bufs=1)
            nc.vector.memset(acc[:], 0.0)
            halfpi = pool.tile([128, 1], F32, tag="halfpi", bufs=1)
            nc.vector.memset(halfpi[:], HALF_PI)
            negh = pool.tile([128, 1], F32, tag="negh", bufs=1)
            nc.vector.memset(negh[:], -0.5)
            w0 = {}

            counters = {"pi": 0, "ci": 0, "wi": 0}

            def load_w0():
                w0a = pool.tile([128, 128], F16, tag="w0a", bufs=1)
                nc.sync.dma_start(w0a[:], w0a_d[:])
                w0b = pool.tile([74, 74], F16, tag="w0b", bufs=1)
                nc.sync.dma_start(w0b[:], w0b_d[:])
                w0[0] = w0a
                w0[127] = w0b

            def phase_pass(b, f0, P, lo):
                pi = counters["pi"]
                counters["pi"] += 1
                sl = slice(0, P)
                cm = pool.tile([P, T], F32, tag="in_a", bufs=in_bufs, name=f"cm{pi}")
                nc.sync.dma_start(cm[:], mag_c[b, f0 : f0 + P, :])
                em = pool.tile([P, T], F32, tag="in_b", bufs=in_bufs, name=f"em{pi}")
                nc.sync.dma_start(em[:], mag_e[b, f0 : f0 + P, :])
                if pi == 0:
                    load_w0()
                cp = pool.tile([P, T], F32, tag="in_c", bufs=cp_bufs, name=f"cp{pi}")
                nc.sync.dma_start(cp[:], pha_c[b, f0 : f0 + P, :])

                junk = pool.tile([P, T], F16, tag="junk", bufs=1, name=f"junk{pi}")

                # mag: m = cm - em (Pool), sum m^2 (ACT Square or DVE stt)
                m = pool.tile([P, T], F16, tag="m", name=f"m{pi}")
                nc.gpsimd.tensor_tensor(m[:], cm[:], em[:], OP.subtract)
                if m2_eng[pi] == "act":
                    nc.scalar.activation(
                        junk[sl, :], m[sl, :], AF.Square,
                        accum_out=acc[sl, (c := col("m2", lo, P)) : c + 1],
                    )
                else:
                    nc.vector.scalar_tensor_tensor(
                        junk[sl, :], m[sl, :], 0.0, m[sl, :], OP.bypass, OP.mult,
                        accum_out=acc[sl, (c := col("m2", lo, P)) : c + 1],
                    )

                # round chain: d -> q -> r = RNE(q) -> f = q - r (fp16)
                d = pool.tile([P, T], F16, tag="d", name=f"d{pi}")
                nc.vector.tensor_tensor(d[:], cp[:], em[:], OP.subtract)
                q = pool.tile([P, T], F16, tag="q", name=f"q{pi}")
                nc.vector.tensor_scalar(q[:], d[:], S, None, OP.mult)
                r = pool.tile([P, T], F16, tag="r", name=f"r{pi}")
                if magic1:
                    # one ts2: (q + 1.5*2^23) - 1.5*2^23 with fp32 internal RNE
                    nc.vector.tensor_scalar(r[:], q[:], MAGIC32, -MAGIC32, OP.add, OP.add)
                else:
                    v = pool.tile([P, T], F16, tag="v", name=f"v{pi}")
                    nc.vector.tensor_scalar(v[:], q[:], MAGIC, None, OP.add)
                    nc.vector.tensor_scalar(r[:], v[:], -MAGIC, None, OP.add)
                f = pool.tile([P, T], F16, tag="f", name=f"f{pi}")
                nc.vector.tensor_tensor(f[:], q[:], r[:], OP.subtract)

                # ip: af = |f| (ACT Abs, accum -> ip); cspc: ACT Sin(pi/2 - 2pi af)
                af = pool.tile([P, T], F16, tag="af", name=f"af{pi}")
                nc.scalar.activation(
                    af[sl, :], f[sl, :], AF.Abs,
                    accum_out=acc[sl, (c := col("ip", lo, P)) : c + 1],
                )
                nc.scalar.activation(
                    junk[sl, :], af[sl, :], AF.Sin, bias=halfpi[0:P, :],
                    scale=NEG_TWO_PI,
                    accum_out=acc[sl, (c := col("cos", lo, P)) : c + 1],
                )

                # gd: y = W0 @ f on PE; ACT Abs psum->g16; dist = ||y|-0.5| accum
                g = pool.tile([P, T], F16, tag="g", name=f"g{pi}")
                for h in range(0, T, qg_cols):
                    qg = psum.tile([P, qg_cols], F32, tag="qg", bufs=2, name=f"qg{pi}_{h}")
                    for n0 in range(0, qg_cols, 512):
                        nc.tensor.matmul(
                            qg[:, n0 : n0 + 512], w0[f0][0:P, 0:P],
                            f[:, h + n0 : h + n0 + 512],
                        )
                    nc.scalar.activation(g[sl, h : h + qg_cols], qg[sl, :], AF.Abs)
                if gd2_eng[pi] == "act":
                    nc.scalar.activation(
                        junk[sl, :], g[sl, :], AF.Abs, bias=negh[0:P, :],
                        accum_out=acc[sl, (c := col("gd", lo, P)) : c + 1],
                    )
                else:
                    zg = pool.tile([P, T], F16, tag="e", name=f"zg{pi}")
                    nc.vector.tensor_scalar(zg[sl, :], g[sl, :], -0.5, None, OP.add)
                    nc.vector.tensor_reduce(
                        acc[sl, (c := col("gd", lo, P)) : c + 1], zg[sl, :],
                        axis=mybir.AxisListType.X, op=OP.add,
                        apply_absolute_value=True,
                    )

                # iaf: fd = shifted diff of f; dist(fd) = |fd - RNE(fd)| accum
                fd = pool.tile([P, T], F16, tag="fd", name=f"fd{pi}")
                nc.vector.tensor_copy(fd[sl, 0:1], f[sl, 0:1])
                nc.vector.tensor_tensor(
                    fd[sl, 1:T], f[sl, 0 : T - 1], f[sl, 1:T], OP.subtract
                )
                u = pool.tile([P, T], F16, tag="r", name=f"u{pi}")
                if magic1:
                    nc.vector.tensor_scalar(u[sl, :], fd[sl, :], MAGIC32, -MAGIC32, OP.add, OP.add)
                else:
                    u1 = pool.tile([P, T], F16, tag="v", name=f"u1{pi}")
                    nc.vector.tensor_scalar(u1[sl, :], fd[sl, :], MAGIC, None, OP.add)
                    nc.vector.tensor_scalar(u[sl, :], u1[sl, :], -MAGIC, None, OP.add)
                e = pool.tile([P, T], F16, tag="e", name=f"e{pi}")
                nc.vector.tensor_tensor(e[sl, :], fd[sl, :], u[sl, :], OP.subtract)
                if iaf2_eng[pi] == "act":
                    nc.scalar.activation(
                        junk[sl, :], e[sl, :], AF.Abs,
                        accum_out=acc[sl, (c := col("iaf", lo, P)) : c + 1],
                    )
                else:
                    nc.vector.tensor_reduce(
                        acc[sl, (c := col("iaf", lo, P)) : c + 1], e[sl, :],
                        axis=mybir.AxisListType.X, op=OP.add,
                        apply_absolute_value=True,
                    )

            def com_pass(b, c0, cols):
                ci = counters["ci"]
                counters["ci"] += 1
                cc = pool.tile([COM_ROWS, cols], F32, tag="in_a", bufs=in_bufs, name=f"cc{ci}")
                nc.sync.dma_start(cc[:], com_c[b, :, c0 : c0 + cols])
                ec = pool.tile([COM_ROWS, cols], F32, tag="in_b", bufs=in_bufs, name=f"ec{ci}")
                nc.sync.dma_start(ec[:], com_e[b, :, c0 : c0 + cols])
                cd = pool.tile([COM_ROWS, cols], F16, tag="cd", bufs=4, name=f"cd{ci}")
                nc.gpsimd.tensor_tensor(cd[:], cc[:], ec[:], OP.subtract)
                djunk = pool.tile([COM_ROWS, cols], F16, tag="junk", bufs=1, name=f"cj{ci}")
                if com_eng[ci] == "act":
                    nc.scalar.activation(
                        djunk[:], cd[:], AF.Square,
                        accum_out=acc[:, (c := col("c2")) : c + 1],
                    )
                else:
                    nc.vector.scalar_tensor_tensor(
                        djunk[:], cd[:], 0.0, cd[:], OP.bypass, OP.mult,
                        accum_out=acc[:, (c := col("c2")) : c + 1],
                    )

            def wav_pass(c0, cols, use_dve):
                wi = counters["wi"]
                counters["wi"] += 1
                cw = pool.tile([WAV_ROWS, cols], F32, tag="in_a", bufs=in_bufs, name=f"cw{wi}")
                nc.sync.dma_start(cw[:], wav_c[:, c0 : c0 + cols])
                ew = pool.tile([WAV_ROWS, cols], F32, tag="in_b", bufs=in_bufs, name=f"ew{wi}")
                nc.sync.dma_start(ew[:], wav_e[:, c0 : c0 + cols])
                wd = pool.tile([WAV_ROWS, cols], F16, tag="cd", bufs=4, name=f"wd{wi}")
                if use_dve:
                    nc.vector.tensor_tensor(wd[:], cw[:], ew[:], OP.subtract)
                else:
                    nc.gpsimd.tensor_tensor(wd[:], cw[:], ew[:], OP.subtract)
                if wav_acc_eng[wi] == "act":
                    wjunk = pool.tile([WAV_ROWS, cols], F16, tag="junk", bufs=1, name=f"wj{wi}")
                    nc.scalar.activation(
                        wjunk[:], wd[:], AF.Abs,
                        accum_out=acc[:, (c := col("w")) : c + 1],
                    )
                else:
                    nc.vector.tensor_reduce(
                        acc[:, (c := col("w")) : c + 1], wd[:],
                        axis=mybir.AxisListType.X, op=OP.add,
                        apply_absolute_value=True,
                    )

            phase_list = [(b, f0, P, lo) for b in range(BPC) for f0, P, lo in FTILES]
            com_list = [(b, c0) for b in range(BPC) for c0 in range(0, COM_COLS, CK)]
            if interleave:
                # one com chunk after each phase pass; the rest trail
                ci = 0
                for k, pp in enumerate(phase_list):
                    phase_pass(*pp)
                    if ci < len(com_list):
                        com_pass(*com_list[ci], CK)
                        ci += 1
                while ci < len(com_list):
                    com_pass(*com_list[ci], CK)
                    ci += 1
            else:
                for pp in phase_list:
                    phase_pass(*pp)
                for b, c0 in com_list:
                    com_pass(b, c0, CK)
            nchunks = len(wav_chunks)
            c0 = 0
            for i, cols in enumerate(wav_chunks):
                wav_pass(c0, cols, last_wav_dve and i == nchunks - 1)
                c0 += cols
            assert c0 == WAV_COLS

            nc.sync.dma_start(out_d[:], acc[:])

    nc.compile()
    return nc


_CACHE = {}


def _get_nc():
    if "nc" not in _CACHE:
        _CACHE["nc"] = build_nc()
    return _CACHE["nc"]


def make_in_maps(inputs):
    """Slice the full inputs into per-core input maps."""
    clean_mag = np.asarray(inputs["clean_mag"], dtype=np.float32)
    enhan_mag = np.asarray(inputs["enhan_mag"], dtype=np.float32)
    clean_pha = np.asarray(inputs["clean_pha"], dtype=np.float32)
    clean_com = np.asarray(inputs["clean_com"], dtype=np.float32)
    enhan_com = np.asarray(inputs["enhan_com"], dtype=np.float32)
    clean_wav = np.asarray(inputs["clean_wav"], dtype=np.float32)
    enhan_wav = np.asarray(inputs["enhan_wav"], dtype=np.float32)

    in_maps = []
    for i in range(NCORES):
        sl = slice(BPC * i, BPC * (i + 1))
        in_maps.append(
            {
                "mag_c": np.ascontiguousarray(clean_mag[sl]),
                "mag_e": np.ascontiguousarray(enhan_mag[sl]),
                "pha_c": np.ascontiguousarray(clean_pha[sl]),
                "com_c": np.ascontiguousarray(clean_com[sl]).reshape(
                    BPC, COM_ROWS, COM_COLS
                ),
                "com_e": np.ascontiguousarray(enhan_com[sl]).reshape(
                    BPC, COM_ROWS, COM_COLS
                ),
                "wav_c": np.ascontiguousarray(clean_wav[sl]).reshape(
                    WAV_ROWS, WAV_COLS
                ),
                "wav_e": np.ascontiguousarray(enhan_wav[sl]).reshape(
                    WAV_ROWS, WAV_COLS
                ),
            }
        )
    return in_maps


def combine(partials, inputs):
    """Combine per-core [128, NCOLS] partial sums into the 6 losses."""
    p = np.asarray(partials, dtype=np.float64)
    p = p.reshape(-1, 128, NCOLS)  # [cores, partitions, cols]

    def tsum(term):
        return sum(p[:, lo:hi, c].sum() for (c, lo, hi) in COLMAP[term])

    s_ip = tsum("ip")
    s_gd = tsum("gd")
    s_iaf = tsum("iaf")
    s_cos = tsum("cos")
    s_m2 = tsum("m2")
    s_c2 = tsum("c2")
    s_w = tsum("w")

    n = float(B * F * T)
    ip = TWO_PI_64 * s_ip / n
    # gd col holds sum(||y|-0.5|); dist(y) = 0.5 - ||y|-0.5|
    gd = TWO_PI_64 * (0.5 * n - s_gd) / n
    # iaf col holds sum(|fd - RNE(fd)|) = sum dist(fd) directly
    iaf = TWO_PI_64 * s_iaf / n
    cspc = 1.0 - s_cos / n
    loss_mag = s_m2 / n
    loss_pha = ip + gd + iaf + cspc
    loss_com = 2.0 * s_c2 / (n * 2.0)
    loss_time = s_w / float(B * L)

    metric_g = np.asarray(inputs["metric_g"], dtype=np.float64).reshape(-1)
    one_labels = np.asarray(inputs["one_labels"], dtype=np.float64).reshape(-1)
    loss_metric = float(np.mean((metric_g - one_labels) ** 2))

    nloss = (
        loss_mag * 0.9
        + loss_pha * 0.3
        + loss_com * 0.1
        + loss_metric * 0.05
        + loss_time * 0.2
    )
    return tuple(
        np.float32(x)
        for x in (nloss, loss_mag, loss_pha, loss_com, loss_metric, loss_time)
    )


def _get_runner():
    """Build (once) a persistently-compiled 8-core sharded executor.

    Mirrors bass2jax.run_bass_via_pjrt but caches the jitted function so
    repeat calls skip retracing/recompiling.
    """
    if "runner" in _CACHE:
        return _CACHE["runner"]
    import jax
    from concourse import bass2jax

    nc = _get_nc()
    bass2jax.install_neuronx_cc_hook()

    partition_name = nc.partition_id_tensor.name if nc.partition_id_tensor else None
    in_names, out_names, out_avals, zero_shapes = [], [], [], []
    for alloc in nc.m.functions[0].allocations:
        if not isinstance(alloc, mybir.MemoryLocationSet):
            continue
        name = alloc.memorylocations[0].name
        if alloc.kind == "ExternalInput":
            if name != partition_name:
                in_names.append(name)
        elif alloc.kind == "ExternalOutput":
            out_names.append(name)
            shape = tuple(alloc.tensor_shape)
            dtype = mybir.dt.np(alloc.dtype)
            out_avals.append(jax.core.ShapedArray(shape, dtype))
            zero_shapes.append((shape, dtype))
    n_params = len(in_names)
    all_in = list(in_names) + list(out_names)
    if partition_name is not None:
        all_in.append(partition_name)
    donate = tuple(range(n_params, n_params + len(out_names)))

    def _body(*args):
        operands = list(args)
        if partition_name is not None:
            operands.append(bass2jax.partition_id_tensor())
        outs = bass2jax._bass_exec_p.bind(
            *operands,
            out_avals=tuple(out_avals),
            in_names=tuple(all_in),
            out_names=tuple(out_names),
            lowering_input_output_aliases=(),
            sim_require_finite=True,
            sim_require_nnan=True,
            nc=nc,
        )
        return tuple(outs)

    devices = jax.devices()[:NCORES]
    mesh = bass2jax.Mesh(np.asarray(devices), ("core",))
    pspec = bass2jax.PartitionSpec("core")
    in_specs = (pspec,) * (n_params + len(out_names))
    out_specs = (pspec,) * len(out_names)
    sharded = jax.jit(
        bass2jax.shard_map(
            _body, mesh=mesh, in_specs=in_specs, out_specs=out_specs, check_rep=False
        ),
        donate_argnums=donate,
        keep_unused=True,
    )

    def make_zeros():
        return [
            np.zeros((NCORES * s[0], *s[1:]), d) for (s, d) in zero_shapes
        ]

    def call(concat_in):
        outs = sharded(*concat_in, *make_zeros())
        return np.asarray(outs[0]).reshape(NCORES, 128, NCOLS)

    def device_put(concat_in):
        sh = jax.sharding.NamedSharding(mesh, pspec)
        return [jax.device_put(a, sh) for a in concat_in]

    runner = (call, in_names, device_put, sharded, make_zeros)
    _CACHE["runner"] = runner
    return runner


def concat_inputs(in_maps, in_names):
    return [
        np.concatenate([m[name] for m in in_maps], axis=0) for name in in_names
    ]


def run(inputs):
    in_maps = make_in_maps(inputs)
    try:
        call, in_names, _, _, _ = _get_runner()
        partials = call(concat_inputs(in_maps, in_names))
    except Exception:
        nc = _get_nc()
        res = run_bass_kernel_spmd(nc, in_maps, core_ids=list(range(NCORES)))
        partials = np.asarray([r["partials"] for r in res.results])
    return combine(partials, inputs)


def kernel(**inputs):
    return run(inputs)


# revision 7
# speedup vs baseline: 1.1694x; 1.0600x over previous
"""Trainium2 Bass kernel for the speech-enhancement loss function.

Math (matching the jax reference):
  loss_mag    = mean((clean_mag - enhan_mag)^2)
  d           = clean_pha - enhan_mag          (reference quirk: enhan_mag is phase_g)
  ip_loss     = mean(aw(d)),   aw(x) = |x - round(x/2pi)*2pi| = 2pi*|f|,
                f = q - round(q), q = d/2pi
  gd_loss     = mean(aw(gd)),  gd[:,0,:] = -d[:,0,:]; gd[:,j,:] = d[:,j-1,:]-d[:,j,:]
  iaf_loss    = mean(aw(iaf)), same shifted difference along the T axis
  cspc_loss   = mean(1 - cos(aw(d))) = mean(1 - cos(2pi f))
  loss_com    = mean((clean_com - enhan_com)^2) * 2
  loss_time   = mean(|clean_wav - enhan_wav|)
  loss_metric = mean((metric_g - 1)^2)            (tiny -> host)

Sharding: data-parallel over the batch dim, 2 batches per core on 8 cores.
Each core computes partial per-partition SUMS of each term into a [128, NCOLS]
accumulator, DMAed out whole; the host reduces partitions + cores.

Device pipeline (per phase tile, fp16 intermediates -- tolerance is 2e-2):
  d16 = cp - em (DVE), q16 = d*S, v16 = q + 1536, r16 = v - 1536 (exact fp16
  round-to-nearest of q), f16 = q - r in [-0.5, 0.5].
  ip:   af16 = |f| via tensor_scalar(abs_max 0) with fused accum (DVE 4x)
  cspc: sin(pi/2 - 2pi*af) via ACT Sin with accum
  gd:   y = W0 @ f16 on PE (fp16, 1 cyc/row); ACT Abs psum->g16; DVE
        ts2((g-0.5) abs_max 0) with accum = sum ||y|-0.5|
  iaf:  fd16 = shifted diff (DVE tt 2x); z16 = (|fd|-0.5) ts2 4x; DVE
        ts(abs_max 0) accum
  m2/c2: Pool subtract -> DVE stt self-mult with accum (1x, proven path)
  wav:  Pool subtract -> DVE ts(abs_max) accum
F-tiling uses OVERLAPPED tiles (rows 0:128 and 127:201): the second tile
recomputes row 127's f locally so its gd needs no cross-tile boundary patch;
its W0 has the j=0 column zeroed and all its accumulations skip row 0.
No final on-device reduction: the [128, NCOLS] acc is DMAed out directly.
Each accum_out instruction OVERWRITES its column, so every accumulation site
has a unique column (COLMAP).

Schedule: 4 phase passes first (their DVE/ACT work overlaps the later com/wav
DMA), com then wav trailing, with a small DVE-only final wav chunk so the
post-DMA tail is ~1us of compute + the fixed DMA-out epilogue. Predicted
~79us vs the 73.4us DMA-transfer floor (26.4 MB/core at 360 GB/s).
"""

import numpy as np

import concourse.bacc as bacc
import concourse.mybir as mybir
import concourse.tile as tile
from concourse.bass_utils import run_bass_kernel_spmd

F32 = mybir.dt.float32
F16 = mybir.dt.float16
OP = mybir.AluOpType
AF = mybir.ActivationFunctionType

B, F, T, L = 16, 201, 2048, 204800
NCORES = 8
BPC = B // NCORES  # batches per core

TWO_PI_64 = 2.0 * np.pi
S = float(np.float32(1.0) / np.float32(TWO_PI_64))  # 1/(2pi)
MAGIC = 1536.0  # 1.5 * 2^10: fp16 round-to-nearest-int trick
MAGIC32 = float(np.float32(1.5 * 2**23))  # fp32 magic (one-op ts2 variant)
HALF_PI = float(np.float32(np.pi / 2))
NEG_TWO_PI = float(np.float32(-TWO_PI_64))

# com per core: BPC*F*T*2 = 1646592 = 2 batches x (128 x 6432)
COM_ROWS, COM_COLS = 128, 6432
COM_CHUNK = 1608  # 4 chunks per batch
# wav per core: BPC*L = 409600 = 128 x 3200
WAV_ROWS, WAV_COLS = 128, 3200

NCOLS = 40  # accumulator columns (one per accumulation instruction)

# term -> list of acc columns, populated by build_nc (deterministic)
COLMAP = {}

# overlapped F tiles: (f0, P, lo) -- accumulate rows [lo:P] of the tile
FTILES = [(0, 128, 0), (127, 74, 1)]


def _w0_matrix(P, skip_first):
    # lhsT[k, j] = delta_{j,k+1} - delta_{j,k}  ->  (W0 @ f)[j] = f[j-1] - f[j]
    # skip_first: zero the j=0 column (row handled by the previous tile)
    w = np.zeros((P, P), dtype=np.float16)
    for k in range(P):
        w[k, k] = -1.0
        if k + 1 < P:
            w[k, k + 1] = 1.0
    if skip_first:
        w[0, 0] = 0.0
    return w


def build_nc(in_bufs=4, cp_bufs=3, wav_chunks=(1408, 1408, 384),
             m2_eng=("act", "act", "act", "act"),
             gd2_eng=("act", "act", "act", "act"),
             iaf2_eng=("dve", "dve", "dve", "dve"),
             com_eng=("dve", "act", "dve", "act", "dve", "act", "dve", "act"),
             wav_acc_eng=("dve", "act", "dve"),
             magic1=True, last_wav_dve=True, qg_cols=1024,
             interleave=True, com_chunk=COM_CHUNK):
    CK = com_chunk
    nc = bacc.Bacc(None, target_bir_lowering=False)

    mag_c = nc.dram_tensor("mag_c", [BPC, F, T], F32, kind="ExternalInput")
    mag_e = nc.dram_tensor("mag_e", [BPC, F, T], F32, kind="ExternalInput")
    pha_c = nc.dram_tensor("pha_c", [BPC, F, T], F32, kind="ExternalInput")
    com_c = nc.dram_tensor("com_c", [BPC, COM_ROWS, COM_COLS], F32, kind="ExternalInput")
    com_e = nc.dram_tensor("com_e", [BPC, COM_ROWS, COM_COLS], F32, kind="ExternalInput")
    wav_c = nc.dram_tensor("wav_c", [WAV_ROWS, WAV_COLS], F32, kind="ExternalInput")
    wav_e = nc.dram_tensor("wav_e", [WAV_ROWS, WAV_COLS], F32, kind="ExternalInput")
    out_d = nc.dram_tensor("partials", [128, NCOLS], F32, kind="ExternalOutput")

    w0a_d = nc.inline_tensor(_w0_matrix(128, False), name="w0a")
    w0b_d = nc.inline_tensor(_w0_matrix(74, True), name="w0b")

    COLMAP.clear()
    _next_col = [0]

    def col(term, lo=0, hi=128):
        c = _next_col[0]
        _next_col[0] += 1
        assert c < NCOLS
        COLMAP.setdefault(term, []).append((c, lo, hi))
        return c

    with tile.TileContext(nc) as tc:
        with (
            tc.tile_pool(name="main", bufs=2) as pool,
            tc.tile_pool(name="psum", bufs=1, space="PSUM") as psum,
        ):
            acc = pool.tile([128, NCOLS], F32, tag="acc", bufs=1)
            nc.vector.memset(acc[:], 0.0)
            halfpi = pool.tile([128, 1], F32, tag="halfpi", bufs=1)
            nc.vector.memset(halfpi[:], HALF_PI)
            negh = pool.tile([128, 1], F32, tag="negh", bufs=1)
            nc.vector.memset(negh[:], -0.5)
            w0 = {}

            counters = {"pi": 0, "ci": 0, "wi": 0}

            def load_w0():
                w0a = pool.tile([128, 128], F16, tag="w0a", bufs=1)
                nc.sync.dma_start(w0a[:], w0a_d[:])
                w0b = pool.tile([74, 74], F16, tag="w0b", bufs=1)
                nc.sync.dma_start(w0b[:], w0b_d[:])
                w0[0] = w0a
                w0[127] = w0b

            def phase_pass(b, f0, P, lo):
                pi = counters["pi"]
                counters["pi"] += 1
                sl = slice(0, P)
                cm = pool.tile([P, T], F32, tag="in_a", bufs=in_bufs, name=f"cm{pi}")
                nc.sync.dma_start(cm[:], mag_c[b, f0 : f0 + P, :])
                em = pool.tile([P, T], F32, tag="in_b", bufs=in_bufs, name=f"em{pi}")
                nc.sync.dma_start(em[:], mag_e[b, f0 : f0 + P, :])
                if pi == 0:
                    load_w0()
                cp = pool.tile([P, T], F32, tag="in_c", bufs=cp_bufs, name=f"cp{pi}")
                nc.sync.dma_start(cp[:], pha_c[b, f0 : f0 + P, :])

                junka = pool.tile([P, T], F16, tag="junka", bufs=2, name=f"junka{pi}")
                junkd = pool.tile([P, T], F16, tag="junkd", bufs=2, name=f"junkd{pi}")

                # mag: m = cm - em (Pool), sum m^2 (ACT Square or DVE stt)
                m = pool.tile([P, T], F16, tag="m", name=f"m{pi}")
                nc.gpsimd.tensor_tensor(m[:], cm[:], em[:], OP.subtract)
                if m2_eng[pi] == "act":
                    nc.scalar.activation(
                        junka[sl, :], m[sl, :], AF.Square,
                        accum_out=acc[sl, (c := col("m2", lo, P)) : c + 1],
                    )
                else:
                    nc.vector.scalar_tensor_tensor(
                        junkd[sl, :], m[sl, :], 0.0, m[sl, :], OP.bypass, OP.mult,
                        accum_out=acc[sl, (c := col("m2", lo, P)) : c + 1],
                    )

                # round chain: d -> q -> r = RNE(q) -> f = q - r (fp16)
                d = pool.tile([P, T], F16, tag="d", name=f"d{pi}")
                nc.vector.tensor_tensor(d[:], cp[:], em[:], OP.subtract)
                q = pool.tile([P, T], F16, tag="q", name=f"q{pi}")
                nc.vector.tensor_scalar(q[:], d[:], S, None, OP.mult)
                r = pool.tile([P, T], F16, tag="r", name=f"r{pi}")
                if magic1:
                    # one ts2: (q + 1.5*2^23) - 1.5*2^23 with fp32 internal RNE
                    nc.vector.tensor_scalar(r[:], q[:], MAGIC32, -MAGIC32, OP.add, OP.add)
                else:
                    v = pool.tile([P, T], F16, tag="v", name=f"v{pi}")
                    nc.vector.tensor_scalar(v[:], q[:], MAGIC, None, OP.add)
                    nc.vector.tensor_scalar(r[:], v[:], -MAGIC, None, OP.add)
                f = pool.tile([P, T], F16, tag="f", name=f"f{pi}")
                nc.vector.tensor_tensor(f[:], q[:], r[:], OP.subtract)

                # ip: af = |f| (ACT Abs, accum -> ip); cspc: ACT Sin(pi/2 - 2pi af)
                af = pool.tile([P, T], F16, tag="af", name=f"af{pi}")
                nc.scalar.activation(
                    af[sl, :], f[sl, :], AF.Abs,
                    accum_out=acc[sl, (c := col("ip", lo, P)) : c + 1],
                )
                nc.scalar.activation(
                    junka[sl, :], af[sl, :], AF.Sin, bias=halfpi[0:P, :],
                    scale=NEG_TWO_PI,
                    accum_out=acc[sl, (c := col("cos", lo, P)) : c + 1],
                )

                # gd: y = W0 @ f on PE; ACT Abs psum->g16; dist = ||y|-0.5| accum
                g = pool.tile([P, T], F16, tag="g", name=f"g{pi}")
                for h in range(0, T, qg_cols):
                    qg = psum.tile([P, qg_cols], F32, tag="qg", bufs=2, name=f"qg{pi}_{h}")
                    for n0 in range(0, qg_cols, 512):
                        nc.tensor.matmul(
                            qg[:, n0 : n0 + 512], w0[f0][0:P, 0:P],
                            f[:, h + n0 : h + n0 + 512],
                        )
                    nc.scalar.activation(g[sl, h : h + qg_cols], qg[sl, :], AF.Abs)
                if gd2_eng[pi] == "act":
                    nc.scalar.activation(
                        junka[sl, :], g[sl, :], AF.Abs, bias=negh[0:P, :],
                        accum_out=acc[sl, (c := col("gd", lo, P)) : c + 1],
                    )
                else:
                    zg = pool.tile([P, T], F16, tag="e", name=f"zg{pi}")
                    nc.vector.tensor_scalar(zg[sl, :], g[sl, :], -0.5, None, OP.add)
                    nc.vector.tensor_reduce(
                        acc[sl, (c := col("gd", lo, P)) : c + 1], zg[sl, :],
                        axis=mybir.AxisListType.X, op=OP.add,
                        apply_absolute_value=True,
                    )

                # iaf: fd = shifted diff of f; dist(fd) = |fd - RNE(fd)| accum
                fd = pool.tile([P, T], F16, tag="fd", name=f"fd{pi}")
                nc.vector.tensor_copy(fd[sl, 0:1], f[sl, 0:1])
                nc.vector.tensor_tensor(
                    fd[sl, 1:T], f[sl, 0 : T - 1], f[sl, 1:T], OP.subtract
                )
                u = pool.tile([P, T], F16, tag="r", name=f"u{pi}")
                if magic1:
                    nc.vector.tensor_scalar(u[sl, :], fd[sl, :], MAGIC32, -MAGIC32, OP.add, OP.add)
                else:
                    u1 = pool.tile([P, T], F16, tag="v", name=f"u1{pi}")
                    nc.vector.tensor_scalar(u1[sl, :], fd[sl, :], MAGIC, None, OP.add)
                    nc.vector.tensor_scalar(u[sl, :], u1[sl, :], -MAGIC, None, OP.add)
                e = pool.tile([P, T], F16, tag="e", name=f"e{pi}")
                nc.vector.tensor_tensor(e[sl, :], fd[sl, :], u[sl, :], OP.subtract)
                if iaf2_eng[pi] == "act":
                    nc.scalar.activation(
                        junka[sl, :], e[sl, :], AF.Abs,
                        accum_out=acc[sl, (c := col("iaf", lo, P)) : c + 1],
                    )
                else:
                    nc.vector.tensor_reduce(
                        acc[sl, (c := col("iaf", lo, P)) : c + 1], e[sl, :],
                        axis=mybir.AxisListType.X, op=OP.add,
                        apply_absolute_value=True,
                    )

            def com_pass(b, c0, cols):
                ci = counters["ci"]
                counters["ci"] += 1
                cc = pool.tile([COM_ROWS, cols], F32, tag="in_a", bufs=in_bufs, name=f"cc{ci}")
                nc.sync.dma_start(cc[:], com_c[b, :, c0 : c0 + cols])
                ec = pool.tile([COM_ROWS, cols], F32, tag="in_b", bufs=in_bufs, name=f"ec{ci}")
                nc.sync.dma_start(ec[:], com_e[b, :, c0 : c0 + cols])
                cd = pool.tile([COM_ROWS, cols], F16, tag="cd", bufs=4, name=f"cd{ci}")
                nc.gpsimd.tensor_tensor(cd[:], cc[:], ec[:], OP.subtract)
                tag = "junka" if com_eng[ci] == "act" else "junkd"
                djunk = pool.tile([COM_ROWS, cols], F16, tag=tag, bufs=2, name=f"cj{ci}")
                if com_eng[ci] == "act":
                    nc.scalar.activation(
                        djunk[:], cd[:], AF.Square,
                        accum_out=acc[:, (c := col("c2")) : c + 1],
                    )
                else:
                    nc.vector.scalar_tensor_tensor(
                        djunk[:], cd[:], 0.0, cd[:], OP.bypass, OP.mult,
                        accum_out=acc[:, (c := col("c2")) : c + 1],
                    )

            def wav_pass(c0, cols, use_dve):
                wi = counters["wi"]
                counters["wi"] += 1
                cw = pool.tile([WAV_ROWS, cols], F32, tag="in_a", bufs=in_bufs, name=f"cw{wi}")
                nc.sync.dma_start(cw[:], wav_c[:, c0 : c0 + cols])
                ew = pool.tile([WAV_ROWS, cols], F32, tag="in_b", bufs=in_bufs, name=f"ew{wi}")
                nc.sync.dma_start(ew[:], wav_e[:, c0 : c0 + cols])
                wd = pool.tile([WAV_ROWS, cols], F16, tag="cd", bufs=4, name=f"wd{wi}")
                if use_dve:
                    nc.vector.tensor_tensor(wd[:], cw[:], ew[:], OP.subtract)
                else:
                    nc.gpsimd.tensor_tensor(wd[:], cw[:], ew[:], OP.subtract)
                if wav_acc_eng[wi] == "act":
                    wjunk = pool.tile([WAV_ROWS, cols], F16, tag="junka", bufs=2, name=f"wj{wi}")
                    nc.scalar.activation(
                        wjunk[:], wd[:], AF.Abs,
                        accum_out=acc[:, (c := col("w")) : c + 1],
                    )
                else:
                    nc.vector.tensor_reduce(
                        acc[:, (c := col("w")) : c + 1], wd[:],
                        axis=mybir.AxisListType.X, op=OP.add,
                        apply_absolute_value=True,
                    )

            phase_list = [(b, f0, P, lo) for b in range(BPC) for f0, P, lo in FTILES]
            com_list = [(b, c0) for b in range(BPC) for c0 in range(0, COM_COLS, CK)]
            if interleave:
                # one com chunk after each phase pass; the rest trail
                ci = 0
                for k, pp in enumerate(phase_list):
                    phase_pass(*pp)
                    if ci < len(com_list):
                        com_pass(*com_list[ci], CK)
                        ci += 1
                while ci < len(com_list):
                    com_pass(*com_list[ci], CK)
                    ci += 1
            else:
                for pp in phase_list:
                    phase_pass(*pp)
                for b, c0 in com_list:
                    com_pass(b, c0, CK)
            nchunks = len(wav_chunks)
            c0 = 0
            for i, cols in enumerate(wav_chunks):
                wav_pass(c0, cols, last_wav_dve and i == nchunks - 1)
                c0 += cols
            assert c0 == WAV_COLS

            nc.sync.dma_start(out_d[:], acc[:])

    nc.compile()
    return nc


_CACHE = {}


def _get_nc():
    if "nc" not in _CACHE:
        _CACHE["nc"] = build_nc()
    return _CACHE["nc"]


def make_in_maps(inputs):
    """Slice the full inputs into per-core input maps."""
    clean_mag = np.asarray(inputs["clean_mag"], dtype=np.float32)
    enhan_mag = np.asarray(inputs["enhan_mag"], dtype=np.float32)
    clean_pha = np.asarray(inputs["clean_pha"], dtype=np.float32)
    clean_com = np.asarray(inputs["clean_com"], dtype=np.float32)
    enhan_com = np.asarray(inputs["enhan_com"], dtype=np.float32)
    clean_wav = np.asarray(inputs["clean_wav"], dtype=np.float32)
    enhan_wav = np.asarray(inputs["enhan_wav"], dtype=np.float32)

    in_maps = []
    for i in range(NCORES):
        sl = slice(BPC * i, BPC * (i + 1))
        in_maps.append(
            {
                "mag_c": np.ascontiguousarray(clean_mag[sl]),
                "mag_e": np.ascontiguousarray(enhan_mag[sl]),
                "pha_c": np.ascontiguousarray(clean_pha[sl]),
                "com_c": np.ascontiguousarray(clean_com[sl]).reshape(
                    BPC, COM_ROWS, COM_COLS
                ),
                "com_e": np.ascontiguousarray(enhan_com[sl]).reshape(
                    BPC, COM_ROWS, COM_COLS
                ),
                "wav_c": np.ascontiguousarray(clean_wav[sl]).reshape(
                    WAV_ROWS, WAV_COLS
                ),
                "wav_e": np.ascontiguousarray(enhan_wav[sl]).reshape(
                    WAV_ROWS, WAV_COLS
                ),
            }
        )
    return in_maps


def combine(partials, inputs):
    """Combine per-core [128, NCOLS] partial sums into the 6 losses."""
    p = np.asarray(partials, dtype=np.float64)
    p = p.reshape(-1, 128, NCOLS)  # [cores, partitions, cols]

    def tsum(term):
        return sum(p[:, lo:hi, c].sum() for (c, lo, hi) in COLMAP[term])

    s_ip = tsum("ip")
    s_gd = tsum("gd")
    s_iaf = tsum("iaf")
    s_cos = tsum("cos")
    s_m2 = tsum("m2")
    s_c2 = tsum("c2")
    s_w = tsum("w")

    n = float(B * F * T)
    ip = TWO_PI_64 * s_ip / n
    # gd col holds sum(||y|-0.5|); dist(y) = 0.5 - ||y|-0.5|
    gd = TWO_PI_64 * (0.5 * n - s_gd) / n
    # iaf col holds sum(|fd - RNE(fd)|) = sum dist(fd) directly
    iaf = TWO_PI_64 * s_iaf / n
    cspc = 1.0 - s_cos / n
    loss_mag = s_m2 / n
    loss_pha = ip + gd + iaf + cspc
    loss_com = 2.0 * s_c2 / (n * 2.0)
    loss_time = s_w / float(B * L)

    metric_g = np.asarray(inputs["metric_g"], dtype=np.float64).reshape(-1)
    one_labels = np.asarray(inputs["one_labels"], dtype=np.float64).reshape(-1)
    loss_metric = float(np.mean((metric_g - one_labels) ** 2))

    nloss = (
        loss_mag * 0.9
        + loss_pha * 0.3
        + loss_com * 0.1
        + loss_metric * 0.05
        + loss_time * 0.2
    )
    return tuple(
        np.float32(x)
        for x in (nloss, loss_mag, loss_pha, loss_com, loss_metric, loss_time)
    )


def _get_runner():
    """Build (once) a persistently-compiled 8-core sharded executor.

    Mirrors bass2jax.run_bass_via_pjrt but caches the jitted function so
    repeat calls skip retracing/recompiling.
    """
    if "runner" in _CACHE:
        return _CACHE["runner"]
    import jax
    from concourse import bass2jax

    nc = _get_nc()
    bass2jax.install_neuronx_cc_hook()

    partition_name = nc.partition_id_tensor.name if nc.partition_id_tensor else None
    in_names, out_names, out_avals, zero_shapes = [], [], [], []
    for alloc in nc.m.functions[0].allocations:
        if not isinstance(alloc, mybir.MemoryLocationSet):
            continue
        name = alloc.memorylocations[0].name
        if alloc.kind == "ExternalInput":
            if name != partition_name:
                in_names.append(name)
        elif alloc.kind == "ExternalOutput":
            out_names.append(name)
            shape = tuple(alloc.tensor_shape)
            dtype = mybir.dt.np(alloc.dtype)
            out_avals.append(jax.core.ShapedArray(shape, dtype))
            zero_shapes.append((shape, dtype))
    n_params = len(in_names)
    all_in = list(in_names) + list(out_names)
    if partition_name is not None:
        all_in.append(partition_name)
    donate = tuple(range(n_params, n_params + len(out_names)))

    def _body(*args):
        operands = list(args)
        if partition_name is not None:
            operands.append(bass2jax.partition_id_tensor())
        outs = bass2jax._bass_exec_p.bind(
            *operands,
            out_avals=tuple(out_avals),
            in_names=tuple(all_in),
            out_names=tuple(out_names),
            lowering_input_output_aliases=(),
            sim_require_finite=True,
            sim_require_nnan=True,
            nc=nc,
        )
        return tuple(outs)

    devices = jax.devices()[:NCORES]
    mesh = bass2jax.Mesh(np.asarray(devices), ("core",))
    pspec = bass2jax.PartitionSpec("core")
    in_specs = (pspec,) * (n_params + len(out_names))
    out_specs = (pspec,) * len(out_names)
    sharded = jax.jit(
        bass2jax.shard_map(
            _body, mesh=mesh, in_specs=in_specs, out_specs=out_specs, check_rep=False
        ),
        donate_argnums=donate,
        keep_unused=True,
    )

    def make_zeros():
        return [
            np.zeros((NCORES * s[0], *s[1:]), d) for (s, d) in zero_shapes
        ]

    def call(concat_in):
        outs = sharded(*concat_in, *make_zeros())
        return np.asarray(outs[0]).reshape(NCORES, 128, NCOLS)

    def device_put(concat_in):
        sh = jax.sharding.NamedSharding(mesh, pspec)
        return [jax.device_put(a, sh) for a in concat_in]

    runner = (call, in_names, device_put, sharded, make_zeros)
    _CACHE["runner"] = runner
    return runner


def concat_inputs(in_maps, in_names):
    return [
        np.concatenate([m[name] for m in in_maps], axis=0) for name in in_names
    ]


def run(inputs):
    in_maps = make_in_maps(inputs)
    try:
        call, in_names, _, _, _ = _get_runner()
        partials = call(concat_inputs(in_maps, in_names))
    except Exception:
        nc = _get_nc()
        res = run_bass_kernel_spmd(nc, in_maps, core_ids=list(range(NCORES)))
        partials = np.asarray([r["partials"] for r in res.results])
    return combine(partials, inputs)


def kernel(**inputs):
    return run(inputs)


# revision 8
# speedup vs baseline: 1.1930x; 1.0202x over previous
"""Trainium2 Bass kernel for the speech-enhancement loss function.

Math (matching the jax reference):
  loss_mag    = mean((clean_mag - enhan_mag)^2)
  d           = clean_pha - enhan_mag          (reference quirk: enhan_mag is phase_g)
  ip_loss     = mean(aw(d)),   aw(x) = |x - round(x/2pi)*2pi| = 2pi*|f|,
                f = q - round(q), q = d/2pi
  gd_loss     = mean(aw(gd)),  gd[:,0,:] = -d[:,0,:]; gd[:,j,:] = d[:,j-1,:]-d[:,j,:]
  iaf_loss    = mean(aw(iaf)), same shifted difference along the T axis
  cspc_loss   = mean(1 - cos(aw(d))) = mean(1 - cos(2pi f))
  loss_com    = mean((clean_com - enhan_com)^2) * 2
  loss_time   = mean(|clean_wav - enhan_wav|)
  loss_metric = mean((metric_g - 1)^2)            (tiny -> host)

Sharding: data-parallel over the batch dim, 2 batches per core on 8 cores.
Each core computes partial per-partition SUMS of each term into a [128, NCOLS]
accumulator, DMAed out whole; the host reduces partitions + cores.

Device pipeline (per phase tile, fp16 intermediates -- tolerance is 2e-2):
  d16 = cp - em (DVE), q16 = d*S, v16 = q + 1536, r16 = v - 1536 (exact fp16
  round-to-nearest of q), f16 = q - r in [-0.5, 0.5].
  ip:   af16 = |f| via tensor_scalar(abs_max 0) with fused accum (DVE 4x)
  cspc: sin(pi/2 - 2pi*af) via ACT Sin with accum
  gd:   y = W0 @ f16 on PE (fp16, 1 cyc/row); ACT Abs psum->g16; DVE
        ts2((g-0.5) abs_max 0) with accum = sum ||y|-0.5|
  iaf:  fd16 = shifted diff (DVE tt 2x); z16 = (|fd|-0.5) ts2 4x; DVE
        ts(abs_max 0) accum
  m2/c2: Pool subtract -> DVE stt self-mult with accum (1x, proven path)
  wav:  Pool subtract -> DVE ts(abs_max) accum
F-tiling uses OVERLAPPED tiles (rows 0:128 and 127:201): the second tile
recomputes row 127's f locally so its gd needs no cross-tile boundary patch;
its W0 has the j=0 column zeroed and all its accumulations skip row 0.
No final on-device reduction: the [128, NCOLS] acc is DMAed out directly.
Each accum_out instruction OVERWRITES its column, so every accumulation site
has a unique column (COLMAP).

Schedule: 4 phase passes first (their DVE/ACT work overlaps the later com/wav
DMA), com then wav trailing, with a small DVE-only final wav chunk so the
post-DMA tail is ~1us of compute + the fixed DMA-out epilogue. Predicted
~79us vs the 73.4us DMA-transfer floor (26.4 MB/core at 360 GB/s).
"""

import numpy as np

import concourse.bacc as bacc
import concourse.mybir as mybir
import concourse.tile as tile
from concourse.bass_utils import run_bass_kernel_spmd

F32 = mybir.dt.float32
F16 = mybir.dt.float16
OP = mybir.AluOpType
AF = mybir.ActivationFunctionType

B, F, T, L = 16, 201, 2048, 204800
NCORES = 8
BPC = B // NCORES  # batches per core

TWO_PI_64 = 2.0 * np.pi
S = float(np.float32(1.0) / np.float32(TWO_PI_64))  # 1/(2pi)
MAGIC = 1536.0  # 1.5 * 2^10: fp16 round-to-nearest-int trick
MAGIC32 = float(np.float32(1.5 * 2**23))  # fp32 magic (one-op ts2 variant)
HALF_PI = float(np.float32(np.pi / 2))
NEG_TWO_PI = float(np.float32(-TWO_PI_64))

# com per core: BPC*F*T*2 = 1646592 = 2 batches x (128 x 6432)
COM_ROWS, COM_COLS = 128, 6432
COM_CHUNK = 1608  # 4 chunks per batch
# wav per core: BPC*L = 409600 = 128 x 3200
WAV_ROWS, WAV_COLS = 128, 3200

NCOLS = 40  # accumulator columns (one per accumulation instruction)

# term -> list of acc columns, populated by build_nc (deterministic)
COLMAP = {}

# overlapped F tiles: (f0, P, lo) -- accumulate rows [lo:P] of the tile
FTILES = [(0, 128, 0), (127, 74, 1)]


def _w0_matrix(P, skip_first):
    # lhsT[k, j] = delta_{j,k+1} - delta_{j,k}  ->  (W0 @ f)[j] = f[j-1] - f[j]
    # skip_first: zero the j=0 column (row handled by the previous tile)
    w = np.zeros((P, P), dtype=np.float16)
    for k in range(P):
        w[k, k] = -1.0
        if k + 1 < P:
            w[k, k + 1] = 1.0
    if skip_first:
        w[0, 0] = 0.0
    return w


def build_nc(in_bufs=4, cp_bufs=3, wav_chunks=(1664, 768, 512, 256),
             m2_eng=("act", "act", "act", "act"),
             gd2_eng=("act", "act", "act", "act"),
             iaf2_eng=("dve", "dve", "dve", "dve"),
             com_eng=("dve", "act", "dve", "act", "dve", "act", "dve", "act"),
             wav_acc_eng=("act", "act", "dve", "dve"),
             wav_sub_eng=("pool", "dve", "dve", "dve"),
             magic1=True, qg_cols=1024,
             interleave=True, com_chunk=COM_CHUNK):
    CK = com_chunk
    nc = bacc.Bacc(None, target_bir_lowering=False)

    mag_c = nc.dram_tensor("mag_c", [BPC, F, T], F32, kind="ExternalInput")
    mag_e = nc.dram_tensor("mag_e", [BPC, F, T], F32, kind="ExternalInput")
    pha_c = nc.dram_tensor("pha_c", [BPC, F, T], F32, kind="ExternalInput")
    com_c = nc.dram_tensor("com_c", [BPC, COM_ROWS, COM_COLS], F32, kind="ExternalInput")
    com_e = nc.dram_tensor("com_e", [BPC, COM_ROWS, COM_COLS], F32, kind="ExternalInput")
    wav_c = nc.dram_tensor("wav_c", [WAV_ROWS, WAV_COLS], F32, kind="ExternalInput")
    wav_e = nc.dram_tensor("wav_e", [WAV_ROWS, WAV_COLS], F32, kind="ExternalInput")
    out_d = nc.dram_tensor("partials", [128, NCOLS], F32, kind="ExternalOutput")

    w0a_d = nc.inline_tensor(_w0_matrix(128, False), name="w0a")
    w0b_d = nc.inline_tensor(_w0_matrix(74, True), name="w0b")

    COLMAP.clear()
    _next_col = [0]

    def col(term, lo=0, hi=128):
        c = _next_col[0]
        _next_col[0] += 1
        assert c < NCOLS
        COLMAP.setdefault(term, []).append((c, lo, hi))
        return c

    with tile.TileContext(nc) as tc:
        with (
            tc.tile_pool(name="main", bufs=2) as pool,
            tc.tile_pool(name="psum", bufs=1, space="PSUM") as psum,
        ):
            acc = pool.tile([128, NCOLS], F32, tag="acc", bufs=1)
            nc.vector.memset(acc[:], 0.0)
            halfpi = pool.tile([128, 1], F32, tag="halfpi", bufs=1)
            nc.vector.memset(halfpi[:], HALF_PI)
            negh = pool.tile([128, 1], F32, tag="negh", bufs=1)
            nc.vector.memset(negh[:], -0.5)
            w0 = {}

            counters = {"pi": 0, "ci": 0, "wi": 0}

            def load_w0():
                w0a = pool.tile([128, 128], F16, tag="w0a", bufs=1)
                nc.sync.dma_start(w0a[:], w0a_d[:])
                w0b = pool.tile([74, 74], F16, tag="w0b", bufs=1)
                nc.sync.dma_start(w0b[:], w0b_d[:])
                w0[0] = w0a
                w0[127] = w0b

            def phase_pass(b, f0, P, lo):
                pi = counters["pi"]
                counters["pi"] += 1
                sl = slice(0, P)
                cm = pool.tile([P, T], F32, tag="in_a", bufs=in_bufs, name=f"cm{pi}")
                nc.sync.dma_start(cm[:], mag_c[b, f0 : f0 + P, :])
                em = pool.tile([P, T], F32, tag="in_b", bufs=in_bufs, name=f"em{pi}")
                nc.sync.dma_start(em[:], mag_e[b, f0 : f0 + P, :])
                if pi == 0:
                    load_w0()
                cp = pool.tile([P, T], F32, tag="in_c", bufs=cp_bufs, name=f"cp{pi}")
                nc.sync.dma_start(cp[:], pha_c[b, f0 : f0 + P, :])

                junka = pool.tile([P, T], F16, tag="junka", bufs=2, name=f"junka{pi}")
                junkd = pool.tile([P, T], F16, tag="junkd", bufs=2, name=f"junkd{pi}")

                # mag: m = cm - em (Pool), sum m^2 (ACT Square or DVE stt)
                m = pool.tile([P, T], F16, tag="m", name=f"m{pi}")
                nc.gpsimd.tensor_tensor(m[:], cm[:], em[:], OP.subtract)
                if m2_eng[pi] == "act":
                    nc.scalar.activation(
                        junka[sl, :], m[sl, :], AF.Square,
                        accum_out=acc[sl, (c := col("m2", lo, P)) : c + 1],
                    )
                else:
                    nc.vector.scalar_tensor_tensor(
                        junkd[sl, :], m[sl, :], 0.0, m[sl, :], OP.bypass, OP.mult,
                        accum_out=acc[sl, (c := col("m2", lo, P)) : c + 1],
                    )

                # round chain: d -> q -> r = RNE(q) -> f = q - r (fp16)
                d = pool.tile([P, T], F16, tag="d", name=f"d{pi}")
                nc.vector.tensor_tensor(d[:], cp[:], em[:], OP.subtract)
                q = pool.tile([P, T], F16, tag="q", name=f"q{pi}")
                nc.vector.tensor_scalar(q[:], d[:], S, None, OP.mult)
                r = pool.tile([P, T], F16, tag="r", name=f"r{pi}")
                if magic1:
                    # one ts2: (q + 1.5*2^23) - 1.5*2^23 with fp32 internal RNE
                    nc.vector.tensor_scalar(r[:], q[:], MAGIC32, -MAGIC32, OP.add, OP.add)
                else:
                    v = pool.tile([P, T], F16, tag="v", name=f"v{pi}")
                    nc.vector.tensor_scalar(v[:], q[:], MAGIC, None, OP.add)
                    nc.vector.tensor_scalar(r[:], v[:], -MAGIC, None, OP.add)
                f = pool.tile([P, T], F16, tag="f", name=f"f{pi}")
                nc.vector.tensor_tensor(f[:], q[:], r[:], OP.subtract)

                # ip: af = |f| (ACT Abs, accum -> ip); cspc: ACT Sin(pi/2 - 2pi af)
                af = pool.tile([P, T], F16, tag="af", name=f"af{pi}")
                nc.scalar.activation(
                    af[sl, :], f[sl, :], AF.Abs,
                    accum_out=acc[sl, (c := col("ip", lo, P)) : c + 1],
                )
                nc.scalar.activation(
                    junka[sl, :], af[sl, :], AF.Sin, bias=halfpi[0:P, :],
                    scale=NEG_TWO_PI,
                    accum_out=acc[sl, (c := col("cos", lo, P)) : c + 1],
                )

                # gd: y = W0 @ f on PE; ACT Abs psum->g16; dist = ||y|-0.5| accum
                g = pool.tile([P, T], F16, tag="g", name=f"g{pi}")
                for h in range(0, T, qg_cols):
                    qg = psum.tile([P, qg_cols], F32, tag="qg", bufs=2, name=f"qg{pi}_{h}")
                    for n0 in range(0, qg_cols, 512):
                        nc.tensor.matmul(
                            qg[:, n0 : n0 + 512], w0[f0][0:P, 0:P],
                            f[:, h + n0 : h + n0 + 512],
                        )
                    nc.scalar.activation(g[sl, h : h + qg_cols], qg[sl, :], AF.Abs)
                if gd2_eng[pi] == "act":
                    nc.scalar.activation(
                        junka[sl, :], g[sl, :], AF.Abs, bias=negh[0:P, :],
                        accum_out=acc[sl, (c := col("gd", lo, P)) : c + 1],
                    )
                else:
                    zg = pool.tile([P, T], F16, tag="e", name=f"zg{pi}")
                    nc.vector.tensor_scalar(zg[sl, :], g[sl, :], -0.5, None, OP.add)
                    nc.vector.tensor_reduce(
                        acc[sl, (c := col("gd", lo, P)) : c + 1], zg[sl, :],
                        axis=mybir.AxisListType.X, op=OP.add,
                        apply_absolute_value=True,
                    )

                # iaf: fd = shifted diff of f; dist(fd) = |fd - RNE(fd)| accum
                fd = pool.tile([P, T], F16, tag="fd", name=f"fd{pi}")
                nc.vector.tensor_copy(fd[sl, 0:1], f[sl, 0:1])
                nc.vector.tensor_tensor(
                    fd[sl, 1:T], f[sl, 0 : T - 1], f[sl, 1:T], OP.subtract
                )
                u = pool.tile([P, T], F16, tag="r", name=f"u{pi}")
                if magic1:
                    nc.vector.tensor_scalar(u[sl, :], fd[sl, :], MAGIC32, -MAGIC32, OP.add, OP.add)
                else:
                    u1 = pool.tile([P, T], F16, tag="v", name=f"u1{pi}")
                    nc.vector.tensor_scalar(u1[sl, :], fd[sl, :], MAGIC, None, OP.add)
                    nc.vector.tensor_scalar(u[sl, :], u1[sl, :], -MAGIC, None, OP.add)
                e = pool.tile([P, T], F16, tag="e", name=f"e{pi}")
                nc.vector.tensor_tensor(e[sl, :], fd[sl, :], u[sl, :], OP.subtract)
                if iaf2_eng[pi] == "act":
                    nc.scalar.activation(
                        junka[sl, :], e[sl, :], AF.Abs,
                        accum_out=acc[sl, (c := col("iaf", lo, P)) : c + 1],
                    )
                else:
                    nc.vector.tensor_reduce(
                        acc[sl, (c := col("iaf", lo, P)) : c + 1], e[sl, :],
                        axis=mybir.AxisListType.X, op=OP.add,
                        apply_absolute_value=True,
                    )

            def com_pass(b, c0, cols):
                ci = counters["ci"]
                counters["ci"] += 1
                cc = pool.tile([COM_ROWS, cols], F32, tag="in_a", bufs=in_bufs, name=f"cc{ci}")
                nc.sync.dma_start(cc[:], com_c[b, :, c0 : c0 + cols])
                ec = pool.tile([COM_ROWS, cols], F32, tag="in_b", bufs=in_bufs, name=f"ec{ci}")
                nc.sync.dma_start(ec[:], com_e[b, :, c0 : c0 + cols])
                cd = pool.tile([COM_ROWS, cols], F16, tag="cd", bufs=4, name=f"cd{ci}")
                nc.gpsimd.tensor_tensor(cd[:], cc[:], ec[:], OP.subtract)
                tag = "junka" if com_eng[ci] == "act" else "junkd"
                djunk = pool.tile([COM_ROWS, cols], F16, tag=tag, bufs=2, name=f"cj{ci}")
                if com_eng[ci] == "act":
                    nc.scalar.activation(
                        djunk[:], cd[:], AF.Square,
                        accum_out=acc[:, (c := col("c2")) : c + 1],
                    )
                else:
                    nc.vector.scalar_tensor_tensor(
                        djunk[:], cd[:], 0.0, cd[:], OP.bypass, OP.mult,
                        accum_out=acc[:, (c := col("c2")) : c + 1],
                    )

            def wav_pass(c0, cols, use_dve):
                wi = counters["wi"]
                counters["wi"] += 1
                cw = pool.tile([WAV_ROWS, cols], F32, tag="in_a", bufs=in_bufs, name=f"cw{wi}")
                nc.sync.dma_start(cw[:], wav_c[:, c0 : c0 + cols])
                ew = pool.tile([WAV_ROWS, cols], F32, tag="in_b", bufs=in_bufs, name=f"ew{wi}")
                nc.sync.dma_start(ew[:], wav_e[:, c0 : c0 + cols])
                wd = pool.tile([WAV_ROWS, cols], F16, tag="cd", bufs=4, name=f"wd{wi}")
                if use_dve:
                    nc.vector.tensor_tensor(wd[:], cw[:], ew[:], OP.subtract)
                else:
                    nc.gpsimd.tensor_tensor(wd[:], cw[:], ew[:], OP.subtract)
                if wav_acc_eng[wi] == "act":
                    wjunk = pool.tile([WAV_ROWS, cols], F16, tag="junka", bufs=2, name=f"wj{wi}")
                    nc.scalar.activation(
                        wjunk[:], wd[:], AF.Abs,
                        accum_out=acc[:, (c := col("w")) : c + 1],
                    )
                else:
                    nc.vector.tensor_reduce(
                        acc[:, (c := col("w")) : c + 1], wd[:],
                        axis=mybir.AxisListType.X, op=OP.add,
                        apply_absolute_value=True,
                    )

            phase_list = [(b, f0, P, lo) for b in range(BPC) for f0, P, lo in FTILES]
            com_list = [(b, c0) for b in range(BPC) for c0 in range(0, COM_COLS, CK)]
            if interleave:
                # one com chunk after each phase pass; the rest trail
                ci = 0
                for k, pp in enumerate(phase_list):
                    phase_pass(*pp)
                    if ci < len(com_list):
                        com_pass(*com_list[ci], CK)
                        ci += 1
                while ci < len(com_list):
                    com_pass(*com_list[ci], CK)
                    ci += 1
            else:
                for pp in phase_list:
                    phase_pass(*pp)
                for b, c0 in com_list:
                    com_pass(b, c0, CK)
            c0 = 0
            for i, cols in enumerate(wav_chunks):
                wav_pass(c0, cols, wav_sub_eng[i] == "dve")
                c0 += cols
            assert c0 == WAV_COLS

            nc.sync.dma_start(out_d[:], acc[:])

    nc.compile()
    return nc


_CACHE = {}


def _get_nc():
    if "nc" not in _CACHE:
        _CACHE["nc"] = build_nc()
    return _CACHE["nc"]


def make_in_maps(inputs):
    """Slice the full inputs into per-core input maps."""
    clean_mag = np.asarray(inputs["clean_mag"], dtype=np.float32)
    enhan_mag = np.asarray(inputs["enhan_mag"], dtype=np.float32)
    clean_pha = np.asarray(inputs["clean_pha"], dtype=np.float32)
    clean_com = np.asarray(inputs["clean_com"], dtype=np.float32)
    enhan_com = np.asarray(inputs["enhan_com"], dtype=np.float32)
    clean_wav = np.asarray(inputs["clean_wav"], dtype=np.float32)
    enhan_wav = np.asarray(inputs["enhan_wav"], dtype=np.float32)

    in_maps = []
    for i in range(NCORES):
        sl = slice(BPC * i, BPC * (i + 1))
        in_maps.append(
            {
                "mag_c": np.ascontiguousarray(clean_mag[sl]),
                "mag_e": np.ascontiguousarray(enhan_mag[sl]),
                "pha_c": np.ascontiguousarray(clean_pha[sl]),
                "com_c": np.ascontiguousarray(clean_com[sl]).reshape(
                    BPC, COM_ROWS, COM_COLS
                ),
                "com_e": np.ascontiguousarray(enhan_com[sl]).reshape(
                    BPC, COM_ROWS, COM_COLS
                ),
                "wav_c": np.ascontiguousarray(clean_wav[sl]).reshape(
                    WAV_ROWS, WAV_COLS
                ),
                "wav_e": np.ascontiguousarray(enhan_wav[sl]).reshape(
                    WAV_ROWS, WAV_COLS
                ),
            }
        )
    return in_maps


def combine(partials, inputs):
    """Combine per-core [128, NCOLS] partial sums into the 6 losses."""
    p = np.asarray(partials, dtype=np.float64)
    p = p.reshape(-1, 128, NCOLS)  # [cores, partitions, cols]

    def tsum(term):
        return sum(p[:, lo:hi, c].sum() for (c, lo, hi) in COLMAP[term])

    s_ip = tsum("ip")
    s_gd = tsum("gd")
    s_iaf = tsum("iaf")
    s_cos = tsum("cos")
    s_m2 = tsum("m2")
    s_c2 = tsum("c2")
    s_w = tsum("w")

    n = float(B * F * T)
    ip = TWO_PI_64 * s_ip / n
    # gd col holds sum(||y|-0.5|); dist(y) = 0.5 - ||y|-0.5|
    gd = TWO_PI_64 * (0.5 * n - s_gd) / n
    # iaf col holds sum(|fd - RNE(fd)|) = sum dist(fd) directly
    iaf = TWO_PI_64 * s_iaf / n
    cspc = 1.0 - s_cos / n
    loss_mag = s_m2 / n
    loss_pha = ip + gd + iaf + cspc
    loss_com = 2.0 * s_c2 / (n * 2.0)
    loss_time = s_w / float(B * L)

    metric_g = np.asarray(inputs["metric_g"], dtype=np.float64).reshape(-1)
    one_labels = np.asarray(inputs["one_labels"], dtype=np.float64).reshape(-1)
    loss_metric = float(np.mean((metric_g - one_labels) ** 2))

    nloss = (
        loss_mag * 0.9
        + loss_pha * 0.3
        + loss_com * 0.1
        + loss_metric * 0.05
        + loss_time * 0.2
    )
    return tuple(
        np.float32(x)
        for x in (nloss, loss_mag, loss_pha, loss_com, loss_metric, loss_time)
    )


def _get_runner():
    """Build (once) a persistently-compiled 8-core sharded executor.

    Mirrors bass2jax.run_bass_via_pjrt but caches the jitted function so
    repeat calls skip retracing/recompiling.
    """
    if "runner" in _CACHE:
        return _CACHE["runner"]
    import jax
    from concourse import bass2jax

    nc = _get_nc()
    bass2jax.install_neuronx_cc_hook()

    partition_name = nc.partition_id_tensor.name if nc.partition_id_tensor else None
    in_names, out_names, out_avals, zero_shapes = [], [], [], []
    for alloc in nc.m.functions[0].allocations:
        if not isinstance(alloc, mybir.MemoryLocationSet):
            continue
        name = alloc.memorylocations[0].name
        if alloc.kind == "ExternalInput":
            if name != partition_name:
                in_names.append(name)
        elif alloc.kind == "ExternalOutput":
            out_names.append(name)
            shape = tuple(alloc.tensor_shape)
            dtype = mybir.dt.np(alloc.dtype)
            out_avals.append(jax.core.ShapedArray(shape, dtype))
            zero_shapes.append((shape, dtype))
    n_params = len(in_names)
    all_in = list(in_names) + list(out_names)
    if partition_name is not None:
        all_in.append(partition_name)
    donate = tuple(range(n_params, n_params + len(out_names)))

    def _body(*args):
        operands = list(args)
        if partition_name is not None:
            operands.append(bass2jax.partition_id_tensor())
        outs = bass2jax._bass_exec_p.bind(
            *operands,
            out_avals=tuple(out_avals),
            in_names=tuple(all_in),
            out_names=tuple(out_names),
            lowering_input_output_aliases=(),
            sim_require_finite=True,
            sim_require_nnan=True,
            nc=nc,
        )
        return tuple(outs)

    devices = jax.devices()[:NCORES]
    mesh = bass2jax.Mesh(np.asarray(devices), ("core",))
    pspec = bass2jax.PartitionSpec("core")
    in_specs = (pspec,) * (n_params + len(out_names))
    out_specs = (pspec,) * len(out_names)
    sharded = jax.jit(
        bass2jax.shard_map(
            _body, mesh=mesh, in_specs=in_specs, out_specs=out_specs, check_rep=False
        ),
        donate_argnums=donate,
        keep_unused=True,
    )

    def make_zeros():
        return [
            np.zeros((NCORES * s[0], *s[1:]), d) for (s, d) in zero_shapes
        ]

    def call(concat_in):
        outs = sharded(*concat_in, *make_zeros())
        return np.asarray(outs[0]).reshape(NCORES, 128, NCOLS)

    def device_put(concat_in):
        sh = jax.sharding.NamedSharding(mesh, pspec)
        return [jax.device_put(a, sh) for a in concat_in]

    runner = (call, in_names, device_put, sharded, make_zeros)
    _CACHE["runner"] = runner
    return runner


def concat_inputs(in_maps, in_names):
    return [
        np.concatenate([m[name] for m in in_maps], axis=0) for name in in_names
    ]


def run(inputs):
    in_maps = make_in_maps(inputs)
    try:
        call, in_names, _, _, _ = _get_runner()
        partials = call(concat_inputs(in_maps, in_names))
    except Exception:
        nc = _get_nc()
        res = run_bass_kernel_spmd(nc, in_maps, core_ids=list(range(NCORES)))
        partials = np.asarray([r["partials"] for r in res.results])
    return combine(partials, inputs)


def kernel(**inputs):
    return run(inputs)


# revision 9
# speedup vs baseline: 1.1990x; 1.0050x over previous
"""Trainium2 Bass kernel for the speech-enhancement loss function.

Math (matching the jax reference):
  loss_mag    = mean((clean_mag - enhan_mag)^2)
  d           = clean_pha - enhan_mag          (reference quirk: enhan_mag is phase_g)
  ip_loss     = mean(aw(d)),   aw(x) = |x - round(x/2pi)*2pi| = 2pi*|f|,
                f = q - round(q), q = d/2pi
  gd_loss     = mean(aw(gd)),  gd[:,0,:] = -d[:,0,:]; gd[:,j,:] = d[:,j-1,:]-d[:,j,:]
  iaf_loss    = mean(aw(iaf)), same shifted difference along the T axis
  cspc_loss   = mean(1 - cos(aw(d))) = mean(1 - cos(2pi f))
  loss_com    = mean((clean_com - enhan_com)^2) * 2
  loss_time   = mean(|clean_wav - enhan_wav|)
  loss_metric = mean((metric_g - 1)^2)            (tiny -> host)

Sharding: data-parallel over the batch dim, 2 batches per core on 8 cores.
Each core computes partial per-partition SUMS of each term into a [128, NCOLS]
accumulator, DMAed out whole; the host reduces partitions + cores.

Device pipeline (per phase tile, fp16 intermediates -- tolerance is 2e-2):
  d16 = cp - em (DVE), q16 = d*S, v16 = q + 1536, r16 = v - 1536 (exact fp16
  round-to-nearest of q), f16 = q - r in [-0.5, 0.5].
  ip:   af16 = |f| via tensor_scalar(abs_max 0) with fused accum (DVE 4x)
  cspc: sin(pi/2 - 2pi*af) via ACT Sin with accum
  gd:   y = W0 @ f16 on PE (fp16, 1 cyc/row); ACT Abs psum->g16; DVE
        ts2((g-0.5) abs_max 0) with accum = sum ||y|-0.5|
  iaf:  fd16 = shifted diff (DVE tt 2x); z16 = (|fd|-0.5) ts2 4x; DVE
        ts(abs_max 0) accum
  m2/c2: Pool subtract -> DVE stt self-mult with accum (1x, proven path)
  wav:  Pool subtract -> DVE ts(abs_max) accum
F-tiling uses OVERLAPPED tiles (rows 0:128 and 127:201): the second tile
recomputes row 127's f locally so its gd needs no cross-tile boundary patch;
its W0 has the j=0 column zeroed and all its accumulations skip row 0.
No final on-device reduction: the [128, NCOLS] acc is DMAed out directly.
Each accum_out instruction OVERWRITES its column, so every accumulation site
has a unique column (COLMAP).

Schedule: 4 phase passes first (their DVE/ACT work overlaps the later com/wav
DMA), com then wav trailing, with a small DVE-only final wav chunk so the
post-DMA tail is ~1us of compute + the fixed DMA-out epilogue. Predicted
~79us vs the 73.4us DMA-transfer floor (26.4 MB/core at 360 GB/s).
"""

import numpy as np

import concourse.bacc as bacc
import concourse.mybir as mybir
import concourse.tile as tile
from concourse.bass_utils import run_bass_kernel_spmd

F32 = mybir.dt.float32
F16 = mybir.dt.float16
OP = mybir.AluOpType
AF = mybir.ActivationFunctionType

B, F, T, L = 16, 201, 2048, 204800
NCORES = 8
BPC = B // NCORES  # batches per core

TWO_PI_64 = 2.0 * np.pi
S = float(np.float32(1.0) / np.float32(TWO_PI_64))  # 1/(2pi)
MAGIC = 1536.0  # 1.5 * 2^10: fp16 round-to-nearest-int trick
MAGIC32 = float(np.float32(1.5 * 2**23))  # fp32 magic (one-op ts2 variant)
HALF_PI = float(np.float32(np.pi / 2))
NEG_TWO_PI = float(np.float32(-TWO_PI_64))

# com per core: BPC*F*T*2 = 1646592 = 2 batches x (128 x 6432)
COM_ROWS, COM_COLS = 128, 6432
COM_CHUNK = 1608  # 4 chunks per batch
# wav per core: BPC*L = 409600 = 128 x 3200
WAV_ROWS, WAV_COLS = 128, 3200

NCOLS = 40  # accumulator columns (one per accumulation instruction)

# term -> list of acc columns, populated by build_nc (deterministic)
COLMAP = {}

# overlapped F tiles: (f0, P, lo) -- accumulate rows [lo:P] of the tile
FTILES = [(0, 128, 0), (127, 74, 1)]


def _w0_matrix(P, skip_first):
    # lhsT[k, j] = delta_{j,k+1} - delta_{j,k}  ->  (W0 @ f)[j] = f[j-1] - f[j]
    # skip_first: zero the j=0 column (row handled by the previous tile)
    w = np.zeros((P, P), dtype=np.float16)
    for k in range(P):
        w[k, k] = -1.0
        if k + 1 < P:
            w[k, k + 1] = 1.0
    if skip_first:
        w[0, 0] = 0.0
    return w


def build_nc(in_bufs=4, cp_bufs=3, wav_chunks=(1664, 768, 512, 256),
             m2_eng=("act", "act", "act", "act"),
             gd2_eng=("act", "act", "act", "act"),
             iaf2_eng=("dve", "dve", "dve", "dve"),
             com_eng=("dve", "act", "dve", "act", "dve", "act", "dve", "act"),
             wav_acc_eng=("act", "act", "dve", "dve"),
             wav_sub_eng=("dve", "dve", "dve", "dve"),
             magic1=True, qg_cols=1024,
             interleave=True, com_chunk=COM_CHUNK):
    CK = com_chunk
    nc = bacc.Bacc(None, target_bir_lowering=False)

    mag_c = nc.dram_tensor("mag_c", [BPC, F, T], F32, kind="ExternalInput")
    mag_e = nc.dram_tensor("mag_e", [BPC, F, T], F32, kind="ExternalInput")
    pha_c = nc.dram_tensor("pha_c", [BPC, F, T], F32, kind="ExternalInput")
    com_c = nc.dram_tensor("com_c", [BPC, COM_ROWS, COM_COLS], F32, kind="ExternalInput")
    com_e = nc.dram_tensor("com_e", [BPC, COM_ROWS, COM_COLS], F32, kind="ExternalInput")
    wav_c = nc.dram_tensor("wav_c", [WAV_ROWS, WAV_COLS], F32, kind="ExternalInput")
    wav_e = nc.dram_tensor("wav_e", [WAV_ROWS, WAV_COLS], F32, kind="ExternalInput")
    out_d = nc.dram_tensor("partials", [128, NCOLS], F32, kind="ExternalOutput")

    w0a_d = nc.inline_tensor(_w0_matrix(128, False), name="w0a")
    w0b_d = nc.inline_tensor(_w0_matrix(74, True), name="w0b")

    COLMAP.clear()
    _next_col = [0]

    def col(term, lo=0, hi=128):
        c = _next_col[0]
        _next_col[0] += 1
        assert c < NCOLS
        COLMAP.setdefault(term, []).append((c, lo, hi))
        return c

    with tile.TileContext(nc) as tc:
        with (
            tc.tile_pool(name="main", bufs=2) as pool,
            tc.tile_pool(name="psum", bufs=1, space="PSUM") as psum,
        ):
            acc = pool.tile([128, NCOLS], F32, tag="acc", bufs=1)
            nc.vector.memset(acc[:], 0.0)
            halfpi = pool.tile([128, 1], F32, tag="halfpi", bufs=1)
            nc.vector.memset(halfpi[:], HALF_PI)
            negh = pool.tile([128, 1], F32, tag="negh", bufs=1)
            nc.vector.memset(negh[:], -0.5)
            w0 = {}

            counters = {"pi": 0, "ci": 0, "wi": 0}

            def load_w0():
                w0a = pool.tile([128, 128], F16, tag="w0a", bufs=1)
                nc.sync.dma_start(w0a[:], w0a_d[:])
                w0b = pool.tile([74, 74], F16, tag="w0b", bufs=1)
                nc.sync.dma_start(w0b[:], w0b_d[:])
                w0[0] = w0a
                w0[127] = w0b

            def phase_pass(b, f0, P, lo):
                pi = counters["pi"]
                counters["pi"] += 1
                sl = slice(0, P)
                cm = pool.tile([P, T], F32, tag="in_a", bufs=in_bufs, name=f"cm{pi}")
                nc.sync.dma_start(cm[:], mag_c[b, f0 : f0 + P, :])
                em = pool.tile([P, T], F32, tag="in_b", bufs=in_bufs, name=f"em{pi}")
                nc.sync.dma_start(em[:], mag_e[b, f0 : f0 + P, :])
                if pi == 0:
                    load_w0()
                cp = pool.tile([P, T], F32, tag="in_c", bufs=cp_bufs, name=f"cp{pi}")
                nc.sync.dma_start(cp[:], pha_c[b, f0 : f0 + P, :])

                junka = pool.tile([P, T], F16, tag="junka", bufs=2, name=f"junka{pi}")
                junkd = pool.tile([P, T], F16, tag="junkd", bufs=2, name=f"junkd{pi}")

                # mag: m = cm - em (Pool), sum m^2 (ACT Square or DVE stt)
                m = pool.tile([P, T], F16, tag="m", name=f"m{pi}")
                nc.gpsimd.tensor_tensor(m[:], cm[:], em[:], OP.subtract)
                if m2_eng[pi] == "act":
                    nc.scalar.activation(
                        junka[sl, :], m[sl, :], AF.Square,
                        accum_out=acc[sl, (c := col("m2", lo, P)) : c + 1],
                    )
                else:
                    nc.vector.scalar_tensor_tensor(
                        junkd[sl, :], m[sl, :], 0.0, m[sl, :], OP.bypass, OP.mult,
                        accum_out=acc[sl, (c := col("m2", lo, P)) : c + 1],
                    )

                # round chain: d -> q -> r = RNE(q) -> f = q - r (fp16)
                d = pool.tile([P, T], F16, tag="d", name=f"d{pi}")
                nc.vector.tensor_tensor(d[:], cp[:], em[:], OP.subtract)
                q = pool.tile([P, T], F16, tag="q", name=f"q{pi}")
                nc.vector.tensor_scalar(q[:], d[:], S, None, OP.mult)
                r = pool.tile([P, T], F16, tag="r", name=f"r{pi}")
                if magic1:
                    # one ts2: (q + 1.5*2^23) - 1.5*2^23 with fp32 internal RNE
                    nc.vector.tensor_scalar(r[:], q[:], MAGIC32, -MAGIC32, OP.add, OP.add)
                else:
                    v = pool.tile([P, T], F16, tag="v", name=f"v{pi}")
                    nc.vector.tensor_scalar(v[:], q[:], MAGIC, None, OP.add)
                    nc.vector.tensor_scalar(r[:], v[:], -MAGIC, None, OP.add)
                f = pool.tile([P, T], F16, tag="f", name=f"f{pi}")
                nc.vector.tensor_tensor(f[:], q[:], r[:], OP.subtract)

                # ip: af = |f| (ACT Abs, accum -> ip); cspc: ACT Sin(pi/2 - 2pi af)
                af = pool.tile([P, T], F16, tag="af", name=f"af{pi}")
                nc.scalar.activation(
                    af[sl, :], f[sl, :], AF.Abs,
                    accum_out=acc[sl, (c := col("ip", lo, P)) : c + 1],
                )
                nc.scalar.activation(
                    junka[sl, :], af[sl, :], AF.Sin, bias=halfpi[0:P, :],
                    scale=NEG_TWO_PI,
                    accum_out=acc[sl, (c := col("cos", lo, P)) : c + 1],
                )

                # gd: y = W0 @ f on PE; ACT Abs psum->g16; dist = ||y|-0.5| accum
                g = pool.tile([P, T], F16, tag="g", name=f"g{pi}")
                for h in range(0, T, qg_cols):
                    qg = psum.tile([P, qg_cols], F32, tag="qg", bufs=2, name=f"qg{pi}_{h}")
                    for n0 in range(0, qg_cols, 512):
                        nc.tensor.matmul(
                            qg[:, n0 : n0 + 512], w0[f0][0:P, 0:P],
                            f[:, h + n0 : h + n0 + 512],
                        )
                    nc.scalar.activation(g[sl, h : h + qg_cols], qg[sl, :], AF.Abs)
                if gd2_eng[pi] == "act":
                    nc.scalar.activation(
                        junka[sl, :], g[sl, :], AF.Abs, bias=negh[0:P, :],
                        accum_out=acc[sl, (c := col("gd", lo, P)) : c + 1],
                    )
                else:
                    zg = pool.tile([P, T], F16, tag="e", name=f"zg{pi}")
                    nc.vector.tensor_scalar(zg[sl, :], g[sl, :], -0.5, None, OP.add)
                    nc.vector.tensor_reduce(
                        acc[sl, (c := col("gd", lo, P)) : c + 1], zg[sl, :],
                        axis=mybir.AxisListType.X, op=OP.add,
                        apply_absolute_value=True,
                    )

                # iaf: fd = shifted diff of f; dist(fd) = |fd - RNE(fd)| accum
                fd = pool.tile([P, T], F16, tag="fd", name=f"fd{pi}")
                nc.vector.tensor_copy(fd[sl, 0:1], f[sl, 0:1])
                nc.vector.tensor_tensor(
                    fd[sl, 1:T], f[sl, 0 : T - 1], f[sl, 1:T], OP.subtract
                )
                u = pool.tile([P, T], F16, tag="r", name=f"u{pi}")
                if magic1:
                    nc.vector.tensor_scalar(u[sl, :], fd[sl, :], MAGIC32, -MAGIC32, OP.add, OP.add)
                else:
                    u1 = pool.tile([P, T], F16, tag="v", name=f"u1{pi}")
                    nc.vector.tensor_scalar(u1[sl, :], fd[sl, :], MAGIC, None, OP.add)
                    nc.vector.tensor_scalar(u[sl, :], u1[sl, :], -MAGIC, None, OP.add)
                e = pool.tile([P, T], F16, tag="e", name=f"e{pi}")
                nc.vector.tensor_tensor(e[sl, :], fd[sl, :], u[sl, :], OP.subtract)
                if iaf2_eng[pi] == "act":
                    nc.scalar.activation(
                        junka[sl, :], e[sl, :], AF.Abs,
                        accum_out=acc[sl, (c := col("iaf", lo, P)) : c + 1],
                    )
                else:
                    nc.vector.tensor_reduce(
                        acc[sl, (c := col("iaf", lo, P)) : c + 1], e[sl, :],
                        axis=mybir.AxisListType.X, op=OP.add,
                        apply_absolute_value=True,
                    )

            def com_pass(b, c0, cols):
                ci = counters["ci"]
                counters["ci"] += 1
                cc = pool.tile([COM_ROWS, cols], F32, tag="in_a", bufs=in_bufs, name=f"cc{ci}")
                nc.sync.dma_start(cc[:], com_c[b, :, c0 : c0 + cols])
                ec = pool.tile([COM_ROWS, cols], F32, tag="in_b", bufs=in_bufs, name=f"ec{ci}")
                nc.sync.dma_start(ec[:], com_e[b, :, c0 : c0 + cols])
                cd = pool.tile([COM_ROWS, cols], F16, tag="cd", bufs=4, name=f"cd{ci}")
                nc.gpsimd.tensor_tensor(cd[:], cc[:], ec[:], OP.subtract)
                tag = "junka" if com_eng[ci] == "act" else "junkd"
                djunk = pool.tile([COM_ROWS, cols], F16, tag=tag, bufs=2, name=f"cj{ci}")
                if com_eng[ci] == "act":
                    nc.scalar.activation(
                        djunk[:], cd[:], AF.Square,
                        accum_out=acc[:, (c := col("c2")) : c + 1],
                    )
                else:
                    nc.vector.scalar_tensor_tensor(
                        djunk[:], cd[:], 0.0, cd[:], OP.bypass, OP.mult,
                        accum_out=acc[:, (c := col("c2")) : c + 1],
                    )

            def wav_pass(c0, cols, use_dve):
                wi = counters["wi"]
                counters["wi"] += 1
                cw = pool.tile([WAV_ROWS, cols], F32, tag="in_a", bufs=in_bufs, name=f"cw{wi}")
                nc.sync.dma_start(cw[:], wav_c[:, c0 : c0 + cols])
                ew = pool.tile([WAV_ROWS, cols], F32, tag="in_b", bufs=in_bufs, name=f"ew{wi}")
                nc.sync.dma_start(ew[:], wav_e[:, c0 : c0 + cols])
                wd = pool.tile([WAV_ROWS, cols], F16, tag="cd", bufs=4, name=f"wd{wi}")
                if use_dve:
                    nc.vector.tensor_tensor(wd[:], cw[:], ew[:], OP.subtract)
                else:
                    nc.gpsimd.tensor_tensor(wd[:], cw[:], ew[:], OP.subtract)
                if wav_acc_eng[wi] == "act":
                    wjunk = pool.tile([WAV_ROWS, cols], F16, tag="junka", bufs=2, name=f"wj{wi}")
                    nc.scalar.activation(
                        wjunk[:], wd[:], AF.Abs,
                        accum_out=acc[:, (c := col("w")) : c + 1],
                    )
                else:
                    nc.vector.tensor_reduce(
                        acc[:, (c := col("w")) : c + 1], wd[:],
                        axis=mybir.AxisListType.X, op=OP.add,
                        apply_absolute_value=True,
                    )

            phase_list = [(b, f0, P, lo) for b in range(BPC) for f0, P, lo in FTILES]
            com_list = [(b, c0) for b in range(BPC) for c0 in range(0, COM_COLS, CK)]
            if interleave:
                # one com chunk after each phase pass; the rest trail
                ci = 0
                for k, pp in enumerate(phase_list):
                    phase_pass(*pp)
                    if ci < len(com_list):
                        com_pass(*com_list[ci], CK)
                        ci += 1
                while ci < len(com_list):
                    com_pass(*com_list[ci], CK)
                    ci += 1
            else:
                for pp in phase_list:
                    phase_pass(*pp)
                for b, c0 in com_list:
                    com_pass(b, c0, CK)
            c0 = 0
            for i, cols in enumerate(wav_chunks):
                wav_pass(c0, cols, wav_sub_eng[i] == "dve")
                c0 += cols
            assert c0 == WAV_COLS

            nc.sync.dma_start(out_d[:], acc[:])

    nc.compile()
    return nc


_CACHE = {}


def _get_nc():
    if "nc" not in _CACHE:
        _CACHE["nc"] = build_nc()
    return _CACHE["nc"]


def make_in_maps(inputs):
    """Slice the full inputs into per-core input maps."""
    clean_mag = np.asarray(inputs["clean_mag"], dtype=np.float32)
    enhan_mag = np.asarray(inputs["enhan_mag"], dtype=np.float32)
    clean_pha = np.asarray(inputs["clean_pha"], dtype=np.float32)
    clean_com = np.asarray(inputs["clean_com"], dtype=np.float32)
    enhan_com = np.asarray(inputs["enhan_com"], dtype=np.float32)
    clean_wav = np.asarray(inputs["clean_wav"], dtype=np.float32)
    enhan_wav = np.asarray(inputs["enhan_wav"], dtype=np.float32)

    in_maps = []
    for i in range(NCORES):
        sl = slice(BPC * i, BPC * (i + 1))
        in_maps.append(
            {
                "mag_c": np.ascontiguousarray(clean_mag[sl]),
                "mag_e": np.ascontiguousarray(enhan_mag[sl]),
                "pha_c": np.ascontiguousarray(clean_pha[sl]),
                "com_c": np.ascontiguousarray(clean_com[sl]).reshape(
                    BPC, COM_ROWS, COM_COLS
                ),
                "com_e": np.ascontiguousarray(enhan_com[sl]).reshape(
                    BPC, COM_ROWS, COM_COLS
                ),
                "wav_c": np.ascontiguousarray(clean_wav[sl]).reshape(
                    WAV_ROWS, WAV_COLS
                ),
                "wav_e": np.ascontiguousarray(enhan_wav[sl]).reshape(
                    WAV_ROWS, WAV_COLS
                ),
            }
        )
    return in_maps


def combine(partials, inputs):
    """Combine per-core [128, NCOLS] partial sums into the 6 losses."""
    p = np.asarray(partials, dtype=np.float64)
    p = p.reshape(-1, 128, NCOLS)  # [cores, partitions, cols]

    def tsum(term):
        return sum(p[:, lo:hi, c].sum() for (c, lo, hi) in COLMAP[term])

    s_ip = tsum("ip")
    s_gd = tsum("gd")
    s_iaf = tsum("iaf")
    s_cos = tsum("cos")
    s_m2 = tsum("m2")
    s_c2 = tsum("c2")
    s_w = tsum("w")

    n = float(B * F * T)
    ip = TWO_PI_64 * s_ip / n
    # gd col holds sum(||y|-0.5|); dist(y) = 0.5 - ||y|-0.5|
    gd = TWO_PI_64 * (0.5 * n - s_gd) / n
    # iaf col holds sum(|fd - RNE(fd)|) = sum dist(fd) directly
    iaf = TWO_PI_64 * s_iaf / n
    cspc = 1.0 - s_cos / n
    loss_mag = s_m2 / n
    loss_pha = ip + gd + iaf + cspc
    loss_com = 2.0 * s_c2 / (n * 2.0)
    loss_time = s_w / float(B * L)

    metric_g = np.asarray(inputs["metric_g"], dtype=np.float64).reshape(-1)
    one_labels = np.asarray(inputs["one_labels"], dtype=np.float64).reshape(-1)
    loss_metric = float(np.mean((metric_g - one_labels) ** 2))

    nloss = (
        loss_mag * 0.9
        + loss_pha * 0.3
        + loss_com * 0.1
        + loss_metric * 0.05
        + loss_time * 0.2
    )
    return tuple(
        np.float32(x)
        for x in (nloss, loss_mag, loss_pha, loss_com, loss_metric, loss_time)
    )


def _get_runner():
    """Build (once) a persistently-compiled 8-core sharded executor.

    Mirrors bass2jax.run_bass_via_pjrt but caches the jitted function so
    repeat calls skip retracing/recompiling.
    """
    if "runner" in _CACHE:
        return _CACHE["runner"]
    import jax
    from concourse import bass2jax

    nc = _get_nc()
    bass2jax.install_neuronx_cc_hook()

    partition_name = nc.partition_id_tensor.name if nc.partition_id_tensor else None
    in_names, out_names, out_avals, zero_shapes = [], [], [], []
    for alloc in nc.m.functions[0].allocations:
        if not isinstance(alloc, mybir.MemoryLocationSet):
            continue
        name = alloc.memorylocations[0].name
        if alloc.kind == "ExternalInput":
            if name != partition_name:
                in_names.append(name)
        elif alloc.kind == "ExternalOutput":
            out_names.append(name)
            shape = tuple(alloc.tensor_shape)
            dtype = mybir.dt.np(alloc.dtype)
            out_avals.append(jax.core.ShapedArray(shape, dtype))
            zero_shapes.append((shape, dtype))
    n_params = len(in_names)
    all_in = list(in_names) + list(out_names)
    if partition_name is not None:
        all_in.append(partition_name)
    donate = tuple(range(n_params, n_params + len(out_names)))

    def _body(*args):
        operands = list(args)
        if partition_name is not None:
            operands.append(bass2jax.partition_id_tensor())
        outs = bass2jax._bass_exec_p.bind(
            *operands,
            out_avals=tuple(out_avals),
            in_names=tuple(all_in),
            out_names=tuple(out_names),
            lowering_input_output_aliases=(),
            sim_require_finite=True,
            sim_require_nnan=True,
            nc=nc,
        )
        return tuple(outs)

    devices = jax.devices()[:NCORES]
    mesh = bass2jax.Mesh(np.asarray(devices), ("core",))
    pspec = bass2jax.PartitionSpec("core")
    in_specs = (pspec,) * (n_params + len(out_names))
    out_specs = (pspec,) * len(out_names)
    sharded = jax.jit(
        bass2jax.shard_map(
            _body, mesh=mesh, in_specs=in_specs, out_specs=out_specs, check_rep=False
        ),
        donate_argnums=donate,
        keep_unused=True,
    )

    def make_zeros():
        return [
            np.zeros((NCORES * s[0], *s[1:]), d) for (s, d) in zero_shapes
        ]

    def call(concat_in):
        outs = sharded(*concat_in, *make_zeros())
        return np.asarray(outs[0]).reshape(NCORES, 128, NCOLS)

    def device_put(concat_in):
        sh = jax.sharding.NamedSharding(mesh, pspec)
        return [jax.device_put(a, sh) for a in concat_in]

    runner = (call, in_names, device_put, sharded, make_zeros)
    _CACHE["runner"] = runner
    return runner


def concat_inputs(in_maps, in_names):
    return [
        np.concatenate([m[name] for m in in_maps], axis=0) for name in in_names
    ]


def run(inputs):
    in_maps = make_in_maps(inputs)
    try:
        call, in_names, _, _, _ = _get_runner()
        partials = call(concat_inputs(in_maps, in_names))
    except Exception:
        nc = _get_nc()
        res = run_bass_kernel_spmd(nc, in_maps, core_ids=list(range(NCORES)))
        partials = np.asarray([r["partials"] for r in res.results])
    return combine(partials, inputs)


def kernel(**inputs):
    return run(inputs)
